# revision 27
# baseline (speedup 1.0000x reference)
"""GNN message-passing on 8 trn2 cores — source-sharded + ReduceScatter.

Strategy v2:
  - Nodes are partitioned across 8 cores x 54 chunks x 128 slots (<=116 real
    nodes per chunk, last chunk 102). A node's owner core holds BOTH its
    features (src role) and computes its BN/activation (dst role).
  - Edges are processed on the core that OWNS THE SOURCE: each core gathers
    only from its LOCAL node-major fp16 table (no replication!), reduces each
    global dst chunk with one-hot matmuls in PSUM, and stages fp16 partial
    aggregates [8 shards][128 feat][6912 slots] in DRAM.
  - One ReduceScatter per layer sums the partials and hands each core its own
    shard — out is only 1.77MB so the collective costs ~59us instead of the
    ~251us AllGather of the replicated-table design.
  - Dense transforms (agg@Wr + h@Ws + b) run post-RS on own chunks in fp16;
    BatchNorm stats via the tiny AllGather exchange; BN+ReLU fused on the
    scalar engine; PE transposes rebuild the local node-major table.
"""

import sys

import numpy as np

sys.path.insert(0, "/opt/trn_rl_repo")

import concourse.bass as bass  # noqa: E402
import concourse.mybir as mybir  # noqa: E402
import concourse.tile as tile  # noqa: E402
from concourse.vector_clock import ScopedClock  # noqa: E402
from concourse import library_config  # noqa: E402
from concourse.library_overlay import lower_extended_insts  # noqa: E402
from concourse.tile_rust import add_dep_helper  # noqa: E402

N = 50000
E = 800000
D = 128
L = 3
OUT = 2
EPS = 1e-5
N_CORES = 8
CHUNKS = 54                 # chunks (dst windows of 128 slots) per core
P = 128
FULL_W = 116                # real nodes in chunks 0..52
LAST_W = 102                # real nodes in chunk 53  (53*116 + 102 = 6250)
SLOTS_PER_CORE = CHUNKS * P          # 6912
N_PAD = N_CORES * SLOTS_PER_CORE     # 55296
NBINS = N_CORES * CHUNKS             # 432
GROUP_BINS = 18             # dst bins per dma_gather call
HALF = 27                   # chunks per RS half
PIECES = [(0, 27), (27, 48), (48, 54)]   # RS piece chunk ranges
CAP = 2 * P                 # target edges per (src core, bin)

F16 = mybir.dt.float16
F32 = mybir.dt.float32
I16 = mybir.dt.int16

_MAX_WAITS = 1


def _drain_and_barrier(self, tick_clock, wait_clock):
    nc = self.nc
    drain_inst = nc.sync.drain()
    wait_clock.add_sem_waits(
        drain_inst.ins, ScopedClock({None: tick_clock.global_clock})
    )
    si = drain_inst.ins.sync_info
    if si is not None and si.on_wait is not None and len(si.on_wait) > _MAX_WAITS:
        waits = list(si.on_wait)
        si.on_wait = waits[:_MAX_WAITS]
        rest = waits[_MAX_WAITS:]
        for i in range(0, len(rest), _MAX_WAITS):
            nop = nc.sync.nop(nofuse=True)
            nop.ins.sync_info = mybir.SyncInfo(
                on_wait=rest[i : i + _MAX_WAITS], on_update=[]
            )
    nc.all_engine_barrier()
    assert self.sems is not None
    popped = nc._tile_sem_poison_stack.pop()
    assert popped is self._sem_poison
    nc.clear_and_free_semaphores(list(self.sems.allocated().values()))
    nc.all_engine_barrier()


tile.TileContext._drain_and_barrier = _drain_and_barrier


def _split_multiwait(nc):
    for fn in nc.m.functions:
        for blk in fn.blocks:
            out = []
            for inst in blk.instructions:
                si = inst.sync_info
                if si is not None and si.on_wait and len(si.on_wait) > _MAX_WAITS:
                    waits = list(si.on_wait)
                    si.on_wait = waits[-_MAX_WAITS:]
                    rest = waits[:-_MAX_WAITS]
                    for i in range(0, len(rest), _MAX_WAITS):
                        out.append(
                            mybir.InstNoOp(
                                name=f"{inst.name}-ws{i}",
                                engine=inst.engine,
                                ins=[],
                                outs=[],
                                bass_nofuse=True,
                                sync_info=mybir.SyncInfo(
                                    on_wait=rest[i : i + _MAX_WAITS], on_update=[]
                                ),
                                debug=inst.debug,
                            )
                        )
                out.append(inst)
            blk.instructions[:] = out


# ---------------------------------------------------------------------------
# Host-side graph partitioning
# ---------------------------------------------------------------------------
def _bin_caps():
    caps = np.full(NBINS, FULL_W, np.int64)
    caps[CHUNKS - 1 :: CHUNKS] = LAST_W
    return caps


def _partition_nodes(src, dst):
    """Assign nodes to (bin, slot). Bins balance in-degree (LPT), then a
    repair pass swaps dst nodes between same-core bins so that per
    (src core, bin) edge counts stay <= CAP where possible."""
    rng = np.random.default_rng(7)
    deg = np.bincount(dst, minlength=N)
    caps = _bin_caps()
    order = np.argsort(-deg, kind="stable")
    bin_of = np.empty(N, np.int32)
    fill = np.zeros(NBINS, np.int64)
    sums = np.zeros(NBINS, np.int64)
    import heapq

    heap = [(0, b) for b in range(NBINS)]
    heapq.heapify(heap)
    for node in order:
        d = int(deg[node])
        while True:
            s, b = heapq.heappop(heap)
            if fill[b] < caps[b]:
                break
        bin_of[node] = b
        fill[b] += 1
        sums[b] += d
        if fill[b] < caps[b]:
            heapq.heappush(heap, (int(sums[b]), b))

    # repair: eliminate per (src core, bin) counts over CAP by swapping dst
    # nodes between same-core bins (global best-gain swaps, feasibility
    # checked vectorized over all candidate partners).
    core_of_node = bin_of // CHUNKS
    src_core = core_of_node[src]
    evc = np.zeros((N, N_CORES), np.int64)
    np.add.at(evc, (dst, src_core), 1)
    cnt = np.zeros((N_CORES, NBINS), np.int64)
    np.add.at(cnt, (src_core, bin_of[dst]), 1)
    order_n = np.argsort(bin_of, kind="stable")
    counts_b = np.bincount(bin_of, minlength=NBINS)
    ends_b = np.cumsum(counts_b)
    node_lists = [
        order_n[ends_b[b] - counts_b[b] : ends_b[b]].copy()
        for b in range(NBINS)
    ]
    import time as _time
    t_end = _time.time() + 60.0
    for _round in range(6):
        over = np.argwhere(cnt > CAP)
        if len(over) == 0 or _time.time() > t_end:
            break
        progress = False
        for c, b in over:
            core = b // CHUNKS
            binrange = np.arange(core * CHUNKS, (core + 1) * CHUNKS)
            while cnt[c, b] > CAP and _time.time() < t_end:
                lb = node_lists[b]
                vs = lb[np.argsort(-evc[lb, c])[:6]]
                best = None
                for v in vs:
                    vc = evc[v]
                    if vc[c] == 0:
                        break
                    for b2 in binrange:
                        if b2 == b:
                            continue
                        l2 = node_lists[b2]
                        wcs = evc[l2]
                        gain = vc[c] - wcs[:, c]
                        newb2 = cnt[:, b2][None] + vc[None] - wcs
                        newb = cnt[:, b][None] - vc[None] + wcs
                        ok = ((newb2 <= CAP).all(1)
                              & (newb <= np.maximum(CAP, cnt[:, b])[None])
                              .all(1) & (gain > 0))
                        if ok.any():
                            gm = np.where(ok, gain, -1)
                            i = int(np.argmax(gm))
                            cand = (int(gm[i]), v, b2, l2[i])
                            if best is None or cand[0] > best[0]:
                                best = cand
                if best is None:
                    break
                g, v, b2, w = best
                vc, wc = evc[v], evc[w]
                bin_of[v] = b2
                bin_of[w] = b
                cnt[:, b] += wc - vc
                cnt[:, b2] += vc - wc
                lb2 = node_lists[b2]
                node_lists[b] = np.where(node_lists[b] == v, w, node_lists[b])
                node_lists[b2] = np.where(lb2 == w, v, lb2)
                progress = True
        if not progress:
            break
    # slots within bin: real nodes first
    slot_of = np.empty(N, np.int32)
    for b in range(NBINS):
        nodes_b = np.where(bin_of == b)[0]
        slot_of[nodes_b] = np.arange(len(nodes_b))
    return bin_of, slot_of


def _preprocess(x, edge_index):
    x = np.asarray(x, np.float32)
    ei = np.asarray(edge_index)
    src = ei[0].astype(np.int64)
    dst = ei[1].astype(np.int64)
    bin_of, slot_of = _partition_nodes(src, dst)

    core_of_node = (bin_of // CHUNKS).astype(np.int64)
    chunk_of_node = (bin_of % CHUNKS).astype(np.int64)
    local_row = chunk_of_node * P + slot_of          # row in owner's table
    newid = core_of_node * SLOTS_PER_CORE + local_row

    src_core = core_of_node[src]
    e_bin = bin_of[dst].astype(np.int64)

    # processing order: half-major (chunks 0..26 of every shard first), so the
    # first ReduceScatter piece can overlap the second half's compute.
    proc_bins = np.array(
        [s_ * CHUNKS + j
         for (a, b_) in PIECES for s_ in range(N_CORES) for j in range(a, b_)],
        np.int64)
    pos_of_bin = np.empty(NBINS, np.int64)
    pos_of_bin[proc_bins] = np.arange(NBINS)

    # per (core, bin) counts -> tiles per bin (shared across cores)
    cnt = np.zeros((N_CORES, NBINS), np.int64)
    np.add.at(cnt, (src_core, e_bin), 1)
    TBraw = np.maximum(2, -(-cnt.max(axis=0) // P))  # [NBINS] by raw bin id
    TB = TBraw[proc_bins]                            # by processed position
    tile_off = np.concatenate([[0], np.cumsum(TB)[:-1]])  # by position
    TILES = int(TB.sum())

    # order edges per core by processed bin position
    idx_grids = np.zeros((N_CORES, TILES * P), np.int16)
    dst_grids = np.full((N_CORES, TILES * P), -1.0, np.float16)
    for c in range(N_CORES):
        m = src_core == c
        eb = pos_of_bin[e_bin[m]]
        es = local_row[src[m]]
        ed = slot_of[dst[m]]
        o = np.argsort(eb, kind="stable")
        eb, es, ed = eb[o], es[o], ed[o]
        starts = np.searchsorted(eb, np.arange(NBINS))
        pos_in_bin = np.arange(len(eb)) - starts[eb]
        flat = (tile_off[eb] * P) + pos_in_bin
        assert (pos_in_bin < TB[eb] * P).all()
        idx_grids[c, flat] = es.astype(np.int16)
        dst_grids[c, flat] = ed.astype(np.float16)

    # dst table [128, TILES]: value at (p, t) = dst slot of edge (t, p)
    dst_cores = np.ascontiguousarray(
        dst_grids.reshape(N_CORES, TILES, P).transpose(0, 2, 1)
    ).astype(np.float32)

    # gather calls: groups of GROUP_BINS bins; idx wrapped [i%16, i//16]
    # replicated to 128 partitions, columns contiguous per call.
    calls = []          # (first_pos, npos, ntiles, col_off)
    col_off = 0
    pos = 0
    for (a, b_) in PIECES:
        lim = pos + (b_ - a) * N_CORES
        b0 = pos
        while b0 < lim:
            nb = min(GROUP_BINS, lim - b0)
            ntiles = int(TB[b0 : b0 + nb].sum())
            calls.append((b0, nb, ntiles, col_off))
            col_off += ntiles * P // 16
            b0 += nb
        pos = lim
    I_COLS = col_off
    idx_cores = np.zeros((N_CORES, P, I_COLS), np.int16)
    for c in range(N_CORES):
        off = 0
        for (b0, nb, ntiles, co) in calls:
            t0 = int(tile_off[b0])
            seq = idx_grids[c, t0 * P : (t0 + ntiles) * P]
            w = seq.reshape(-1, 16).T            # [16, n/16]
            idx_cores[c, :, co : co + ntiles * P // 16] = np.tile(w, (8, 1))

    # initial tables
    x_pad = np.zeros((N_PAD, D), np.float32)
    x_pad[newid] = x
    x_loc = np.ascontiguousarray(
        x_pad.reshape(N_CORES, SLOTS_PER_CORE, D).astype(np.float16)
    )
    xT_loc = np.ascontiguousarray(
        x_pad.reshape(N_CORES, SLOTS_PER_CORE, D).transpose(0, 2, 1)
        .astype(np.float16)
    )
    meta = dict(TB=TB, tile_off=tile_off, TILES=TILES, calls=calls,
                I_COLS=I_COLS, proc_bins=proc_bins)
    return meta, newid, idx_cores, dst_cores, x_loc, xT_loc


# ---------------------------------------------------------------------------
# Device program
# ---------------------------------------------------------------------------
def build_program(meta):
    TB = meta["TB"]
    tile_off = meta["tile_off"]
    TILES = meta["TILES"]
    calls = meta["calls"]
    I_COLS = meta["I_COLS"]
    proc_bins = meta["proc_bins"]

    nc = bass.Bass(num_devices=N_CORES)

    p_xloc = nc.declare_dram_parameter("x_loc", [SLOTS_PER_CORE, D], F16,
                                       isOutput=False)
    p_xT = nc.declare_dram_parameter("xT_loc", [D, SLOTS_PER_CORE], F16,
                                     isOutput=False)
    p_idx = nc.declare_dram_parameter("gidx", [P, I_COLS], I16, isOutput=False)
    p_dst = nc.declare_dram_parameter("dst_loc", [P, TILES], F32,
                                      isOutput=False)
    p_wrel = nc.declare_dram_parameter("wrel", [L, D, D], F16, isOutput=False)
    p_wroot = nc.declare_dram_parameter("wroot", [L, D, D], F16, isOutput=False)
    p_w2 = nc.declare_dram_parameter("w2", [D, 2 * OUT], F16, isOutput=False)
    p_bR = nc.declare_dram_parameter("bR", [1, L * D], F16, isOutput=False)
    p_gammaT = nc.declare_dram_parameter("gammaT", [D, L], F32, isOutput=False)
    p_betaT = nc.declare_dram_parameter("betaT", [D, L], F32, isOutput=False)
    p_b2 = nc.declare_dram_parameter("b2", [1, OUT], F16, isOutput=False)
    p_iota = nc.declare_dram_parameter("iota16", [P, P], F16, isOutput=False)
    p_ident = nc.declare_dram_parameter("ident16", [P, P], F16, isOutput=False)
    p_out = nc.declare_dram_parameter("z4T", [OUT, SLOTS_PER_CORE], F16,
                                      isOutput=True)

    rg = [list(range(N_CORES))]
    widths = [FULL_W] * (CHUNKS - 1) + [LAST_W]

    from contextlib import ExitStack
    with tile.TileContext(nc) as tc:
        with ExitStack() as _es:
            dram_zp = _es.enter_context(tc.tile_pool(name="dram_zp", bufs=4, space="DRAM"))
            dram_zo = _es.enter_context(tc.tile_pool(name="dram_zo", bufs=4, space="DRAM"))
            dram_loc = _es.enter_context(tc.tile_pool(name="dram_loc", bufs=2, space="DRAM"))
            dram_cc = _es.enter_context(tc.tile_pool(name="dram_cc", bufs=2, space="DRAM"))
            singles = _es.enter_context(tc.tile_pool(name="singles", bufs=1))
            hT_pool = _es.enter_context(tc.tile_pool(name="hT", bufs=2))
            z_pool = _es.enter_context(tc.tile_pool(name="zb", bufs=1))
            g_pool = _es.enter_context(tc.tile_pool(name="gath", bufs=3))
            s_pool = _es.enter_context(tc.tile_pool(name="sel", bufs=8))
            pair_pool = _es.enter_context(tc.tile_pool(name="pairs", bufs=5))
            aggo_pool = _es.enter_context(tc.tile_pool(name="aggo", bufs=3))
            t16_pool = _es.enter_context(tc.tile_pool(name="t16p", bufs=2))
            bn_pool = _es.enter_context(tc.tile_pool(name="bns", bufs=2))
            stat_pool = _es.enter_context(tc.tile_pool(name="stat", bufs=2))
            psA = _es.enter_context(tc.tile_pool(name="psA", bufs=4, space="PSUM"))
            psZ = _es.enter_context(tc.tile_pool(name="psZ", bufs=2, space="PSUM"))
            psT = _es.enter_context(tc.tile_pool(name="psT", bufs=2, space="PSUM"))
            with tc.high_priority():
                nc.gpsimd.load_library(library_config.mlp)
            call_regs = {}
            for (_, _, ntiles, _) in calls:
                n = ntiles * P
                if n not in call_regs:
                    call_regs[n] = nc.gpsimd.to_reg(n)

            idx_sb = singles.tile([P, I_COLS], I16)
            _c3 = I_COLS // 8
            nc.sync.dma_start(out=idx_sb[:, :_c3], in_=p_idx[:, :_c3])
            nc.sync.dma_start(out=idx_sb[:, _c3 : 3 * _c3],
                              in_=p_idx[:, _c3 : 3 * _c3])
            nc.sync.dma_start(out=idx_sb[:, 3 * _c3 :], in_=p_idx[:, 3 * _c3 :])
            dst_sb = singles.tile([P, TILES], F32)
            nc.sync.dma_start(out=dst_sb[:], in_=p_dst[:])
            iota_sb = singles.tile([P, P], F16)
            nc.sync.dma_start(out=iota_sb[:], in_=p_iota[:])
            ident_sb = singles.tile([P, P], F16)
            nc.sync.dma_start(out=ident_sb[:], in_=p_ident[:])
            wrel_sb = singles.tile([P, L * D], F16)
            wroot_sb = singles.tile([P, L * D], F16)
            for l in range(L):
                nc.sync.dma_start(out=wrel_sb[:, l * D : (l + 1) * D],
                                  in_=p_wrel[l])
                nc.sync.dma_start(out=wroot_sb[:, l * D : (l + 1) * D],
                                  in_=p_wroot[l])
            w2_sb = singles.tile([P, 2 * OUT], F16)
            nc.sync.dma_start(out=w2_sb[:], in_=p_w2[:])
            bR_sb = singles.tile([1, L * D], F16)
            nc.sync.dma_start(out=bR_sb[:], in_=p_bR[:])
            ones_sb = singles.tile([1, P], F16)
            nc.vector.memset(ones_sb[:], 1.0)
            gammaT_sb = singles.tile([P, L], F32)
            nc.sync.dma_start(out=gammaT_sb[:], in_=p_gammaT[:])
            betaT_sb = singles.tile([P, L], F32)
            nc.sync.dma_start(out=betaT_sb[:], in_=p_betaT[:])
            b2_sb = singles.tile([1, OUT], F16)
            nc.sync.dma_start(out=b2_sb[:], in_=p_b2[:])
            eps_sb = singles.tile([P, 1], F32)
            nc.vector.memset(eps_sb[:], EPS)

            hT_prev = hT_pool.tile([P, SLOTS_PER_CORE], F16, tag="hT")
            nc.sync.dma_start(out=hT_prev[:], in_=p_xT[:])
            h_loc = p_xloc

            for l in range(L + 1):
                is_final = l == L
                rows = OUT if is_final else P
                if is_final:
                    zp0 = dram_zp.tile([N_CORES * rows, SLOTS_PER_CORE], F16)
                    zps = [zp0]
                else:
                    zps = [dram_zp.tile([N_CORES * rows, (b_ - a_) * P],
                                        F16, tag="zp_piece", name="zp_piece")
                           for (a_, b_) in PIECES]

                shard_buf = None
                zp_writes = []
                if not is_final:
                    zos = [dram_zo.tile([P, (b_ - a_) * P], F16,
                                        tag="zo_p", name="zo_p")
                           for (a_, b_) in PIECES]
                    piece_wr = [0] * len(PIECES)
                for (p0, nb, ntiles, co) in calls:
                    gath = g_pool.tile([P, ntiles * P], F16, tag="gath")
                    gg = gath.rearrange("p (t d) -> p t d", t=ntiles)
                    nc.gpsimd.dma_gather(
                        out_ap=gg,
                        in_ap=h_loc[:],
                        idxs_ap=idx_sb[:, co : co + ntiles * P // 16],
                        num_idxs=ntiles * P,
                        num_idxs_reg=call_regs[ntiles * P],
                        elem_size=D,
                        single_packet=False,
                    )
                    for pp in range(p0, p0 + nb):
                        b = int(proc_bins[pp])
                        t0 = int(tile_off[pp]) - int(tile_off[p0])
                        tb = int(TB[pp])
                        s_ = b // CHUNKS
                        j = b % CHUNKS
                        pi = next(i for i, (a_, b_) in enumerate(PIECES)
                                  if a_ <= j < b_)
                        pa, pb = PIECES[pi]
                        plen = pb - pa
                        jl = j - pa
                        gi = jl % 4
                        glen = min(4, plen - (jl - gi))
                        if jl == 0:
                            shard_buf = pair_pool.tile([rows, plen * P], F16)
                        sel = s_pool.tile([P, tb * P], F16)
                        for t in range(tb):
                            nc.vector.tensor_scalar(
                                out=sel[:, t * P : (t + 1) * P],
                                in0=iota_sb[:],
                                scalar1=dst_sb[
                                    :, tile_off[pp] + t : tile_off[pp] + t + 1
                                ],
                                scalar2=None,
                                op0=mybir.AluOpType.is_equal,
                            )
                        if gi == 0:
                            ps_a = psA.tile([P, glen * P], F32, space="PSUM")
                        for t in range(tb):
                            nc.tensor.matmul(
                                out=ps_a[:, gi * P : (gi + 1) * P],
                                lhsT=gath[:, (t0 + t) * P : (t0 + t + 1) * P],
                                rhs=sel[:, t * P : (t + 1) * P],
                                start=(t == 0),
                                stop=(t == tb - 1),
                            )
                        if is_final and gi == glen - 1:
                            agg_sb = aggo_pool.tile([P, glen * P], F16)
                            nc.scalar.activation(
                                out=agg_sb[:], in_=ps_a[:],
                                func=mybir.ActivationFunctionType.Copy,
                            )
                            ps2 = psZ.tile([OUT, glen * P], F32, space="PSUM", tag="psz")
                            for g2 in range(glen):
                                nc.tensor.matmul(
                                    out=ps2[:, g2 * P : (g2 + 1) * P],
                                    lhsT=w2_sb[:, :OUT],
                                    rhs=agg_sb[:, g2 * P : (g2 + 1) * P],
                                    start=True, stop=True,
                                )
                            nc.vector.tensor_copy(
                                out=shard_buf[:, (jl - gi) * P : (jl + 1) * P],
                                in_=ps2[:],
                            )
                        elif not is_final and gi == glen - 1:
                            if (jl // 4) % 2 == 0:
                                nc.scalar.activation(
                                    out=shard_buf[:, (jl - gi) * P : (jl + 1) * P],
                                    in_=ps_a[:],
                                    func=mybir.ActivationFunctionType.Copy,
                                )
                            else:
                                nc.vector.tensor_copy(
                                    out=shard_buf[:, (jl - gi) * P : (jl + 1) * P],
                                    in_=ps_a[:],
                                )
                        if jl == plen - 1:
                            if is_final:
                                dstap = bass.AP(
                                    tensor=zps[0].tensor,
                                    offset=zps[0][:].offset
                                    + (s_ * rows) * SLOTS_PER_CORE
                                    + pa * P,
                                    ap=[[SLOTS_PER_CORE, rows], [1, plen * P]],
                                )
                            else:
                                dstap = bass.AP(
                                    tensor=zps[pi].tensor,
                                    offset=zps[pi][:].offset
                                    + (s_ * rows) * (plen * P),
                                    ap=[[plen * P, rows], [1, plen * P]],
                                )
                            zp_w = nc.sync.dma_start(out=dstap,
                                                     in_=shard_buf[:])
                            zp_writes.append(zp_w)
                            if not is_final:
                                piece_wr[pi] += 1
                                if piece_wr[pi] == N_CORES:
                                    bass.BassGpSimd.collective_compute(
                                        nc.sync, "ReduceScatter",
                                        mybir.AluOpType.add,
                                        replica_groups=rg,
                                        ins=[zps[pi].opt()],
                                        outs=[zos[pi].opt()],
                                    )

                # ReduceScatter the partials (two pieces for inner layers:
                # RS of half A overlaps nothing here but runs while the tail
                # of half B compute / copies drain; RS_B follows on the
                # collective device)
                if is_final:
                    zo = dram_zo.tile([OUT, SLOTS_PER_CORE], F16)
                    nc.gpsimd.collective_compute(
                        "ReduceScatter", mybir.AluOpType.add,
                        replica_groups=rg, ins=[zps[0].opt()], outs=[zo.opt()],
                    )


                if is_final:
                    out16 = aggo_pool.tile([OUT, SLOTS_PER_CORE], F16,
                                           tag="aggo")
                    z2_sb = aggo_pool.tile([OUT, SLOTS_PER_CORE], F16,
                                           tag="aggo")
                    z2_ld = nc.sync.dma_start(out=z2_sb[:], in_=zo[:])
                    add_dep_helper(z2_ld.ins, zp_writes[-1].ins,
                                   reason="sp-order: z2 after zp writes")
                    for j in range(CHUNKS):
                        cs = slice(j * P, (j + 1) * P)
                        ps_z = psZ.tile([OUT, P], F32, space="PSUM", tag="psz")
                        nc.tensor.matmul(
                            out=ps_z[:], lhsT=w2_sb[:, OUT : 2 * OUT],
                            rhs=hT_prev[:, cs], start=True, stop=False,
                        )
                        nc.tensor.matmul(
                            out=ps_z[:], lhsT=b2_sb[:], rhs=ones_sb[:],
                            start=False, stop=True,
                        )
                        nc.scalar.activation(
                            out=out16[:, cs], in_=ps_z[:],
                            func=mybir.ActivationFunctionType.Copy,
                        )
                    nc.vector.tensor_tensor(
                        out=out16[:], in0=out16[:], in1=z2_sb[:],
                        op=mybir.AluOpType.add,
                    )
                    nc.sync.dma_start(out=p_out[:], in_=out16[:])
                    continue

                aggTs = []
                for pi, (a_, b_) in enumerate(PIECES):
                    t_agg = aggo_pool.tile([P, (b_ - a_) * P], F16,
                                           tag="aggo", name="aggT_p")
                    agg_ld = nc.sync.dma_start(out=t_agg[:], in_=zos[pi][:])
                    add_dep_helper(agg_ld.ins, zp_writes[-1].ins,
                                   reason="sp-order: aggT after zp writes")
                    aggTs.append(t_agg)
                z_all = z_pool.tile([P, SLOTS_PER_CORE], F16)
                stats = stat_pool.tile([P, CHUNKS, nc.vector.BN_STATS_DIM], F32)
                w_rel = wrel_sb[:, l * D : (l + 1) * D]
                w_root = wroot_sb[:, l * D : (l + 1) * D]
                for j in range(CHUNKS):
                    cs = slice(j * P, (j + 1) * P)
                    pi = next(i for i, (a_, b_) in enumerate(PIECES)
                              if a_ <= j < b_)
                    jrel = j - PIECES[pi][0]
                    agg_src = aggTs[pi][:, jrel * P : (jrel + 1) * P]
                    ps_z = psZ.tile([P, P], F32, space="PSUM", tag="psz")
                    nc.tensor.matmul(out=ps_z[:], lhsT=w_rel, rhs=agg_src,
                                     start=True, stop=False)
                    nc.tensor.matmul(out=ps_z[:], lhsT=w_root,
                                     rhs=hT_prev[:, cs], start=False,
                                     stop=False)
                    nc.tensor.matmul(
                        out=ps_z[:], lhsT=bR_sb[:, l * D : (l + 1) * D],
                        rhs=ones_sb[:], start=False, stop=True,
                    )
                    if j % 2 == 0:
                        nc.scalar.activation(
                            out=z_all[:, cs], in_=ps_z[:],
                            func=mybir.ActivationFunctionType.Copy,
                        )
                    else:
                        nc.vector.tensor_copy(out=z_all[:, cs], in_=ps_z[:])
                    nc.vector.bn_stats(
                        out=stats[:, j, :],
                        in_=z_all[:, j * P : j * P + widths[j]],
                    )

                # ---- BatchNorm over all nodes (mean of per-chunk widths
                # differs, so weight by width via two-group aggregation) ----
                bs = bn_pool.tile([P, 16], F32)
                mv = bs[:, 0:2]
                with tc.high_priority():
                    nc.vector.bn_aggr(out=mv, in_=stats[:, : CHUNKS - 1, :])
                mv2 = bs[:, 13:15]
                with tc.high_priority():
                    nc.vector.bn_aggr(out=mv2, in_=stats[:, CHUNKS - 1 :, :])
                # combine into sums: S1 = nf*mu1 + nl*mu2 ; S2 likewise
                nf = float((CHUNKS - 1) * FULL_W)
                nl = float(LAST_W)
                cc_sb = bs[:, 3:5]
                with tc.high_priority():
                    # cc0 = sum of z = nf*mu_f + nl*mu_l
                    nc.vector.tensor_scalar(
                        out=cc_sb[:, 0:1], in0=mv[:, 0:1], scalar1=nf,
                        scalar2=None, op0=mybir.AluOpType.mult,
                    )
                    nc.vector.tensor_scalar(
                        out=bs[:, 5:6], in0=mv2[:, 0:1], scalar1=nl,
                        scalar2=None, op0=mybir.AluOpType.mult,
                    )
                    nc.vector.tensor_tensor(
                        out=cc_sb[:, 0:1], in0=cc_sb[:, 0:1], in1=bs[:, 5:6],
                        op=mybir.AluOpType.add,
                    )
                    # cc1 = sum of z^2 = nf*(mu_f^2+var_f) + nl*(...)
                    nc.vector.tensor_scalar(
                        out=bs[:, 6:7], in0=mv[:, 0:1], scalar1=mv[:, 0:1],
                        scalar2=mv[:, 1:2], op0=mybir.AluOpType.mult,
                        op1=mybir.AluOpType.add,
                    )
                    nc.vector.tensor_scalar(
                        out=bs[:, 6:7], in0=bs[:, 6:7], scalar1=nf,
                        scalar2=None, op0=mybir.AluOpType.mult,
                    )
                    nc.vector.tensor_scalar(
                        out=bs[:, 7:8], in0=mv2[:, 0:1], scalar1=mv2[:, 0:1],
                        scalar2=mv2[:, 1:2], op0=mybir.AluOpType.mult,
                        op1=mybir.AluOpType.add,
                    )
                    nc.vector.tensor_scalar(
                        out=bs[:, 7:8], in0=bs[:, 7:8], scalar1=nl,
                        scalar2=None, op0=mybir.AluOpType.mult,
                    )
                    nc.vector.tensor_tensor(
                        out=cc_sb[:, 1:2], in0=bs[:, 6:7], in1=bs[:, 7:8],
                        op=mybir.AluOpType.add,
                    )
                cc_in = dram_cc.tile([P, 2], F32)
                cc_out = dram_cc.tile([P * N_CORES, 2], F32,
                                      addr_space="Shared")
                nc.sync.dma_start(out=cc_in[:], in_=cc_sb)
                nc.gpsimd.collective_compute(
                    "AllGather", mybir.AluOpType.bypass, replica_groups=rg,
                    ins=[cc_in.opt()], outs=[cc_out.opt()],
                )
                cc_all = bn_pool.tile([P, 2, N_CORES], F32)
                cc_src = bass.AP(
                    tensor=cc_out.tensor,
                    offset=cc_out[:].offset,
                    ap=[[2, P], [1, 2], [2 * P, N_CORES]],
                )
                nc.sync.dma_start(out=cc_all[:], in_=cc_src)
                cc_res = bs[:, 8:10]
                nc.vector.tensor_reduce(
                    out=cc_res.rearrange("p (a b) -> p a b", a=2),
                    in_=cc_all[:],
                    axis=mybir.AxisListType.X,
                    op=mybir.AluOpType.add,
                )
                mu = bs[:, 10:11]
                nc.vector.tensor_scalar(
                    out=mu, in0=cc_res[:, 0:1], scalar2=None,
                    op0=mybir.AluOpType.mult, scalar1=1.0 / N,
                )
                var = bs[:, 11:12]
                nc.vector.tensor_scalar(
                    out=var, in0=cc_res[:, 1:2], scalar2=None,
                    op0=mybir.AluOpType.mult, scalar1=1.0 / N,
                )
                mu2 = bs[:, 12:13]
                nc.vector.tensor_tensor(
                    out=mu2, in0=mu, in1=mu, op=mybir.AluOpType.mult
                )
                nc.vector.tensor_tensor(
                    out=var, in0=var, in1=mu2, op=mybir.AluOpType.subtract
                )
                rstd = bs[:, 13:14]
                nc.scalar.activation(
                    out=rstd, in_=var,
                    func=mybir.ActivationFunctionType.Sqrt,
                    bias=eps_sb[:], scale=1.0,
                )
                nc.vector.reciprocal(out=rstd, in_=rstd)
                scale = bs[:, 14:15]
                nc.vector.tensor_tensor(
                    out=scale, in0=rstd, in1=gammaT_sb[:, l : l + 1],
                    op=mybir.AluOpType.mult,
                )
                shift = bs[:, 15:16]
                nc.vector.tensor_tensor(
                    out=shift, in0=mu, in1=scale, op=mybir.AluOpType.mult
                )
                nc.vector.tensor_tensor(
                    out=shift, in0=betaT_sb[:, l : l + 1], in1=shift,
                    op=mybir.AluOpType.subtract,
                )

                # BN apply + relu (fp16 out), transpose, rebuild local table
                hT_new = hT_pool.tile([P, SLOTS_PER_CORE], F16, tag="hT")
                loc_new = dram_loc.tile([SLOTS_PER_CORE, D], F16)
                t16g = None
                for j in range(CHUNKS):
                    gi = j % 6
                    if gi == 0:
                        t16g = t16_pool.tile([P, 6, P], F16)
                        gs = slice(j * P, (j + 6) * P)
                        nc.scalar.activation(
                            out=hT_new[:, gs], in_=z_all[:, gs],
                            func=mybir.ActivationFunctionType.Relu,
                            bias=shift, scale=scale,
                        )
                    cs2 = slice(j * P, (j + 1) * P)
                    ps_t = psT.tile([P, P], F16, space="PSUM")
                    nc.tensor.transpose(
                        out=ps_t[:], in_=hT_new[:, cs2], identity=ident_sb[:],
                    )
                    if gi % 2 == 0:
                        nc.vector.tensor_copy(out=t16g[:, gi, :], in_=ps_t[:])
                    else:
                        nc.scalar.activation(
                            out=t16g[:, gi, :], in_=ps_t[:],
                            func=mybir.ActivationFunctionType.Copy,
                        )
                    if gi == 5:
                        g0 = (j - 5) * P
                        dstap = bass.AP(
                            tensor=loc_new.tensor,
                            offset=loc_new[:].offset + g0 * D,
                            ap=[[D, P], [P * D, 6], [1, D]],
                        )
                        nc.sync.dma_start(out=dstap, in_=t16g[:])
                h_loc = loc_new
                hT_prev = hT_new

    lower_extended_insts(nc)
    _split_multiwait(nc)
    return nc


_PROGRAM_CACHE = {}


def _get_program(meta):
    key = (meta["TILES"], meta["I_COLS"], tuple(meta["TB"].tolist()))
    if key not in _PROGRAM_CACHE:
        _PROGRAM_CACHE[key] = build_program(meta)
    return _PROGRAM_CACHE[key]


def _make_in_maps(meta, idx_cores, dst_cores, x_loc, xT_loc,
                  Wrel, Wroot, b, gamma, beta, Wrel2, Wroot2, b2):
    iota16 = np.broadcast_to(np.arange(P, dtype=np.float16), (P, P)).copy()
    ident16 = np.eye(P, dtype=np.float16)
    w2 = np.concatenate(
        [np.asarray(Wrel2, np.float32), np.asarray(Wroot2, np.float32)], axis=1
    ).astype(np.float16)
    common = dict(
        wrel=np.ascontiguousarray(np.asarray(Wrel, np.float32)).astype(np.float16),
        wroot=np.ascontiguousarray(np.asarray(Wroot, np.float32)).astype(np.float16),
        w2=np.ascontiguousarray(w2),
        bR=np.asarray(b, np.float32).reshape(1, L * D).astype(np.float16),
        gammaT=np.ascontiguousarray(np.asarray(gamma, np.float32).T),
        betaT=np.ascontiguousarray(np.asarray(beta, np.float32).T),
        b2=np.asarray(b2, np.float32).reshape(1, OUT).astype(np.float16),
        iota16=iota16,
        ident16=ident16,
    )
    in_maps = []
    for c in range(N_CORES):
        m = dict(common)
        m["x_loc"] = x_loc[c]
        m["xT_loc"] = xT_loc[c]
        m["gidx"] = idx_cores[c]
        m["dst_loc"] = dst_cores[c]
        in_maps.append(m)
    return in_maps


def run(x, edge_index, Wrel, Wroot, b, gamma, beta, Wrel2, Wroot2, b2):
    meta, newid, idx_cores, dst_cores, x_loc, xT_loc = _preprocess(
        x, edge_index)
    nc = _get_program(meta)
    in_maps = _make_in_maps(
        meta, idx_cores, dst_cores, x_loc, xT_loc,
        Wrel, Wroot, b, gamma, beta, Wrel2, Wroot2, b2,
    )
    from concourse.bass_utils import run_bass_kernel_spmd

    res = run_bass_kernel_spmd(nc, in_maps, list(range(N_CORES)))
    full = np.concatenate(
        [res.results[c]["z4T"].T for c in range(N_CORES)], axis=0
    )
    return full[newid].astype(np.float32), nc, meta["TILES"]


def kernel(**inputs):
    out, _, _ = run(**{k: np.asarray(v) for k, v in inputs.items()})
    return out



# revision 28
# speedup vs baseline: 1.0174x; 1.0174x over previous
"""GNN message-passing on 8 trn2 cores — source-sharded + ReduceScatter.

Strategy v2:
  - Nodes are partitioned across 8 cores x 54 chunks x 128 slots (<=116 real
    nodes per chunk, last chunk 102). A node's owner core holds BOTH its
    features (src role) and computes its BN/activation (dst role).
  - Edges are processed on the core that OWNS THE SOURCE: each core gathers
    only from its LOCAL node-major fp16 table (no replication!), reduces each
    global dst chunk with one-hot matmuls in PSUM, and stages fp16 partial
    aggregates [8 shards][128 feat][6912 slots] in DRAM.
  - One ReduceScatter per layer sums the partials and hands each core its own
    shard — out is only 1.77MB so the collective costs ~59us instead of the
    ~251us AllGather of the replicated-table design.
  - Dense transforms (agg@Wr + h@Ws + b) run post-RS on own chunks in fp16;
    BatchNorm stats via the tiny AllGather exchange; BN+ReLU fused on the
    scalar engine; PE transposes rebuild the local node-major table.
"""

import sys

import numpy as np

sys.path.insert(0, "/opt/trn_rl_repo")

import concourse.bass as bass  # noqa: E402
import concourse.mybir as mybir  # noqa: E402
import concourse.tile as tile  # noqa: E402
from concourse.vector_clock import ScopedClock  # noqa: E402
from concourse import library_config  # noqa: E402
from concourse.library_overlay import lower_extended_insts  # noqa: E402
from concourse.tile_rust import add_dep_helper  # noqa: E402

N = 50000
E = 800000
D = 128
L = 3
OUT = 2
EPS = 1e-5
N_CORES = 8
CHUNKS = 54                 # chunks (dst windows of 128 slots) per core
P = 128
FULL_W = 116                # real nodes in chunks 0..52
LAST_W = 102                # real nodes in chunk 53  (53*116 + 102 = 6250)
SLOTS_PER_CORE = CHUNKS * P          # 6912
N_PAD = N_CORES * SLOTS_PER_CORE     # 55296
NBINS = N_CORES * CHUNKS             # 432
GROUP_BINS = 18             # dst bins per dma_gather call
HALF = 27                   # chunks per RS half
PIECES = [(0, 27), (27, 46), (46, 54)]   # RS piece chunk ranges
CAP = 2 * P                 # target edges per (src core, bin)

F16 = mybir.dt.float16
F32 = mybir.dt.float32
I16 = mybir.dt.int16

_MAX_WAITS = 1


def _drain_and_barrier(self, tick_clock, wait_clock):
    nc = self.nc
    drain_inst = nc.sync.drain()
    wait_clock.add_sem_waits(
        drain_inst.ins, ScopedClock({None: tick_clock.global_clock})
    )
    si = drain_inst.ins.sync_info
    if si is not None and si.on_wait is not None and len(si.on_wait) > _MAX_WAITS:
        waits = list(si.on_wait)
        si.on_wait = waits[:_MAX_WAITS]
        rest = waits[_MAX_WAITS:]
        for i in range(0, len(rest), _MAX_WAITS):
            nop = nc.sync.nop(nofuse=True)
            nop.ins.sync_info = mybir.SyncInfo(
                on_wait=rest[i : i + _MAX_WAITS], on_update=[]
            )
    nc.all_engine_barrier()
    assert self.sems is not None
    popped = nc._tile_sem_poison_stack.pop()
    assert popped is self._sem_poison
    nc.clear_and_free_semaphores(list(self.sems.allocated().values()))
    nc.all_engine_barrier()


tile.TileContext._drain_and_barrier = _drain_and_barrier


def _split_multiwait(nc):
    for fn in nc.m.functions:
        for blk in fn.blocks:
            out = []
            for inst in blk.instructions:
                si = inst.sync_info
                if si is not None and si.on_wait and len(si.on_wait) > _MAX_WAITS:
                    waits = list(si.on_wait)
                    si.on_wait = waits[-_MAX_WAITS:]
                    rest = waits[:-_MAX_WAITS]
                    for i in range(0, len(rest), _MAX_WAITS):
                        out.append(
                            mybir.InstNoOp(
                                name=f"{inst.name}-ws{i}",
                                engine=inst.engine,
                                ins=[],
                                outs=[],
                                bass_nofuse=True,
                                sync_info=mybir.SyncInfo(
                                    on_wait=rest[i : i + _MAX_WAITS], on_update=[]
                                ),
                                debug=inst.debug,
                            )
                        )
                out.append(inst)
            blk.instructions[:] = out


# ---------------------------------------------------------------------------
# Host-side graph partitioning
# ---------------------------------------------------------------------------
def _bin_caps():
    caps = np.full(NBINS, FULL_W, np.int64)
    caps[CHUNKS - 1 :: CHUNKS] = LAST_W
    return caps


def _partition_nodes(src, dst):
    """Assign nodes to (bin, slot). Bins balance in-degree (LPT), then a
    repair pass swaps dst nodes between same-core bins so that per
    (src core, bin) edge counts stay <= CAP where possible."""
    rng = np.random.default_rng(7)
    deg = np.bincount(dst, minlength=N)
    caps = _bin_caps()
    order = np.argsort(-deg, kind="stable")
    bin_of = np.empty(N, np.int32)
    fill = np.zeros(NBINS, np.int64)
    sums = np.zeros(NBINS, np.int64)
    import heapq

    heap = [(0, b) for b in range(NBINS)]
    heapq.heapify(heap)
    for node in order:
        d = int(deg[node])
        while True:
            s, b = heapq.heappop(heap)
            if fill[b] < caps[b]:
                break
        bin_of[node] = b
        fill[b] += 1
        sums[b] += d
        if fill[b] < caps[b]:
            heapq.heappush(heap, (int(sums[b]), b))

    # repair: eliminate per (src core, bin) counts over CAP by swapping dst
    # nodes between same-core bins (global best-gain swaps, feasibility
    # checked vectorized over all candidate partners).
    core_of_node = bin_of // CHUNKS
    src_core = core_of_node[src]
    evc = np.zeros((N, N_CORES), np.int64)
    np.add.at(evc, (dst, src_core), 1)
    cnt = np.zeros((N_CORES, NBINS), np.int64)
    np.add.at(cnt, (src_core, bin_of[dst]), 1)
    order_n = np.argsort(bin_of, kind="stable")
    counts_b = np.bincount(bin_of, minlength=NBINS)
    ends_b = np.cumsum(counts_b)
    node_lists = [
        order_n[ends_b[b] - counts_b[b] : ends_b[b]].copy()
        for b in range(NBINS)
    ]
    import time as _time
    t_end = _time.time() + 60.0
    for _round in range(6):
        over = np.argwhere(cnt > CAP)
        if len(over) == 0 or _time.time() > t_end:
            break
        progress = False
        for c, b in over:
            core = b // CHUNKS
            binrange = np.arange(core * CHUNKS, (core + 1) * CHUNKS)
            while cnt[c, b] > CAP and _time.time() < t_end:
                lb = node_lists[b]
                vs = lb[np.argsort(-evc[lb, c])[:6]]
                best = None
                for v in vs:
                    vc = evc[v]
                    if vc[c] == 0:
                        break
                    for b2 in binrange:
                        if b2 == b:
                            continue
                        l2 = node_lists[b2]
                        wcs = evc[l2]
                        gain = vc[c] - wcs[:, c]
                        newb2 = cnt[:, b2][None] + vc[None] - wcs
                        newb = cnt[:, b][None] - vc[None] + wcs
                        ok = ((newb2 <= CAP).all(1)
                              & (newb <= np.maximum(CAP, cnt[:, b])[None])
                              .all(1) & (gain > 0))
                        if ok.any():
                            gm = np.where(ok, gain, -1)
                            i = int(np.argmax(gm))
                            cand = (int(gm[i]), v, b2, l2[i])
                            if best is None or cand[0] > best[0]:
                                best = cand
                if best is None:
                    break
                g, v, b2, w = best
                vc, wc = evc[v], evc[w]
                bin_of[v] = b2
                bin_of[w] = b
                cnt[:, b] += wc - vc
                cnt[:, b2] += vc - wc
                lb2 = node_lists[b2]
                node_lists[b] = np.where(node_lists[b] == v, w, node_lists[b])
                node_lists[b2] = np.where(lb2 == w, v, lb2)
                progress = True
        if not progress:
            break
    # slots within bin: real nodes first
    slot_of = np.empty(N, np.int32)
    for b in range(NBINS):
        nodes_b = np.where(bin_of == b)[0]
        slot_of[nodes_b] = np.arange(len(nodes_b))
    return bin_of, slot_of


def _preprocess(x, edge_index):
    x = np.asarray(x, np.float32)
    ei = np.asarray(edge_index)
    src = ei[0].astype(np.int64)
    dst = ei[1].astype(np.int64)
    bin_of, slot_of = _partition_nodes(src, dst)

    core_of_node = (bin_of // CHUNKS).astype(np.int64)
    chunk_of_node = (bin_of % CHUNKS).astype(np.int64)
    local_row = chunk_of_node * P + slot_of          # row in owner's table
    newid = core_of_node * SLOTS_PER_CORE + local_row

    src_core = core_of_node[src]
    e_bin = bin_of[dst].astype(np.int64)

    # processing order: half-major (chunks 0..26 of every shard first), so the
    # first ReduceScatter piece can overlap the second half's compute.
    proc_bins = np.array(
        [s_ * CHUNKS + j
         for (a, b_) in PIECES for s_ in range(N_CORES) for j in range(a, b_)],
        np.int64)
    pos_of_bin = np.empty(NBINS, np.int64)
    pos_of_bin[proc_bins] = np.arange(NBINS)

    # per (core, bin) counts -> tiles per bin (shared across cores)
    cnt = np.zeros((N_CORES, NBINS), np.int64)
    np.add.at(cnt, (src_core, e_bin), 1)
    TBraw = np.maximum(2, -(-cnt.max(axis=0) // P))  # [NBINS] by raw bin id
    TB = TBraw[proc_bins]                            # by processed position
    tile_off = np.concatenate([[0], np.cumsum(TB)[:-1]])  # by position
    TILES = int(TB.sum())

    # order edges per core by processed bin position
    idx_grids = np.zeros((N_CORES, TILES * P), np.int16)
    dst_grids = np.full((N_CORES, TILES * P), -1.0, np.float16)
    for c in range(N_CORES):
        m = src_core == c
        eb = pos_of_bin[e_bin[m]]
        es = local_row[src[m]]
        ed = slot_of[dst[m]]
        o = np.argsort(eb, kind="stable")
        eb, es, ed = eb[o], es[o], ed[o]
        starts = np.searchsorted(eb, np.arange(NBINS))
        pos_in_bin = np.arange(len(eb)) - starts[eb]
        flat = (tile_off[eb] * P) + pos_in_bin
        assert (pos_in_bin < TB[eb] * P).all()
        idx_grids[c, flat] = es.astype(np.int16)
        dst_grids[c, flat] = ed.astype(np.float16)

    # dst table [128, TILES]: value at (p, t) = dst slot of edge (t, p)
    dst_cores = np.ascontiguousarray(
        dst_grids.reshape(N_CORES, TILES, P).transpose(0, 2, 1)
    ).astype(np.float32)

    # gather calls: groups of GROUP_BINS bins; idx wrapped [i%16, i//16]
    # replicated to 128 partitions, columns contiguous per call.
    calls = []          # (first_pos, npos, ntiles, col_off)
    col_off = 0
    pos = 0
    for (a, b_) in PIECES:
        lim = pos + (b_ - a) * N_CORES
        b0 = pos
        while b0 < lim:
            nb = min(GROUP_BINS, lim - b0)
            ntiles = int(TB[b0 : b0 + nb].sum())
            calls.append((b0, nb, ntiles, col_off))
            col_off += ntiles * P // 16
            b0 += nb
        pos = lim
    I_COLS = col_off
    idx_cores = np.zeros((N_CORES, P, I_COLS), np.int16)
    for c in range(N_CORES):
        off = 0
        for (b0, nb, ntiles, co) in calls:
            t0 = int(tile_off[b0])
            seq = idx_grids[c, t0 * P : (t0 + ntiles) * P]
            w = seq.reshape(-1, 16).T            # [16, n/16]
            idx_cores[c, :, co : co + ntiles * P // 16] = np.tile(w, (8, 1))

    # initial tables
    x_pad = np.zeros((N_PAD, D), np.float32)
    x_pad[newid] = x
    x_loc = np.ascontiguousarray(
        x_pad.reshape(N_CORES, SLOTS_PER_CORE, D).astype(np.float16)
    )
    xT_loc = np.ascontiguousarray(
        x_pad.reshape(N_CORES, SLOTS_PER_CORE, D).transpose(0, 2, 1)
        .astype(np.float16)
    )
    meta = dict(TB=TB, tile_off=tile_off, TILES=TILES, calls=calls,
                I_COLS=I_COLS, proc_bins=proc_bins)
    return meta, newid, idx_cores, dst_cores, x_loc, xT_loc


# ---------------------------------------------------------------------------
# Device program
# ---------------------------------------------------------------------------
def build_program(meta):
    TB = meta["TB"]
    tile_off = meta["tile_off"]
    TILES = meta["TILES"]
    calls = meta["calls"]
    I_COLS = meta["I_COLS"]
    proc_bins = meta["proc_bins"]

    nc = bass.Bass(num_devices=N_CORES)

    p_xloc = nc.declare_dram_parameter("x_loc", [SLOTS_PER_CORE, D], F16,
                                       isOutput=False)
    p_xT = nc.declare_dram_parameter("xT_loc", [D, SLOTS_PER_CORE], F16,
                                     isOutput=False)
    p_idx = nc.declare_dram_parameter("gidx", [P, I_COLS], I16, isOutput=False)
    p_dst = nc.declare_dram_parameter("dst_loc", [P, TILES], F32,
                                      isOutput=False)
    p_wrel = nc.declare_dram_parameter("wrel", [L, D, D], F16, isOutput=False)
    p_wroot = nc.declare_dram_parameter("wroot", [L, D, D], F16, isOutput=False)
    p_w2 = nc.declare_dram_parameter("w2", [D, 2 * OUT], F16, isOutput=False)
    p_bR = nc.declare_dram_parameter("bR", [1, L * D], F16, isOutput=False)
    p_gammaT = nc.declare_dram_parameter("gammaT", [D, L], F32, isOutput=False)
    p_betaT = nc.declare_dram_parameter("betaT", [D, L], F32, isOutput=False)
    p_b2 = nc.declare_dram_parameter("b2", [1, OUT], F16, isOutput=False)
    p_iota = nc.declare_dram_parameter("iota16", [P, P], F16, isOutput=False)
    p_ident = nc.declare_dram_parameter("ident16", [P, P], F16, isOutput=False)
    p_out = nc.declare_dram_parameter("z4T", [OUT, SLOTS_PER_CORE], F16,
                                      isOutput=True)

    rg = [list(range(N_CORES))]
    widths = [FULL_W] * (CHUNKS - 1) + [LAST_W]

    from contextlib import ExitStack
    with tile.TileContext(nc) as tc:
        with ExitStack() as _es:
            dram_zp = _es.enter_context(tc.tile_pool(name="dram_zp", bufs=4, space="DRAM"))
            dram_zo = _es.enter_context(tc.tile_pool(name="dram_zo", bufs=4, space="DRAM"))
            dram_loc = _es.enter_context(tc.tile_pool(name="dram_loc", bufs=2, space="DRAM"))
            dram_cc = _es.enter_context(tc.tile_pool(name="dram_cc", bufs=2, space="DRAM"))
            singles = _es.enter_context(tc.tile_pool(name="singles", bufs=1))
            hT_pool = _es.enter_context(tc.tile_pool(name="hT", bufs=2))
            z_pool = _es.enter_context(tc.tile_pool(name="zb", bufs=1))
            g_pool = _es.enter_context(tc.tile_pool(name="gath", bufs=3))
            s_pool = _es.enter_context(tc.tile_pool(name="sel", bufs=8))
            pair_pool = _es.enter_context(tc.tile_pool(name="pairs", bufs=5))
            aggo_pool = _es.enter_context(tc.tile_pool(name="aggo", bufs=3))
            t16_pool = _es.enter_context(tc.tile_pool(name="t16p", bufs=2))
            bn_pool = _es.enter_context(tc.tile_pool(name="bns", bufs=2))
            stat_pool = _es.enter_context(tc.tile_pool(name="stat", bufs=2))
            psA = _es.enter_context(tc.tile_pool(name="psA", bufs=4, space="PSUM"))
            psZ = _es.enter_context(tc.tile_pool(name="psZ", bufs=2, space="PSUM"))
            psT = _es.enter_context(tc.tile_pool(name="psT", bufs=2, space="PSUM"))
            with tc.high_priority():
                nc.gpsimd.load_library(library_config.mlp)
            call_regs = {}
            for (_, _, ntiles, _) in calls:
                n = ntiles * P
                if n not in call_regs:
                    call_regs[n] = nc.gpsimd.to_reg(n)

            idx_sb = singles.tile([P, I_COLS], I16)
            _c3 = I_COLS // 8
            nc.sync.dma_start(out=idx_sb[:, :_c3], in_=p_idx[:, :_c3])
            nc.sync.dma_start(out=idx_sb[:, _c3 : 3 * _c3],
                              in_=p_idx[:, _c3 : 3 * _c3])
            nc.sync.dma_start(out=idx_sb[:, 3 * _c3 :], in_=p_idx[:, 3 * _c3 :])
            dst_sb = singles.tile([P, TILES], F32)
            nc.sync.dma_start(out=dst_sb[:], in_=p_dst[:])
            iota_sb = singles.tile([P, P], F16)
            nc.sync.dma_start(out=iota_sb[:], in_=p_iota[:])
            ident_sb = singles.tile([P, P], F16)
            nc.sync.dma_start(out=ident_sb[:], in_=p_ident[:])
            wrel_sb = singles.tile([P, L * D], F16)
            wroot_sb = singles.tile([P, L * D], F16)
            for l in range(L):
                nc.sync.dma_start(out=wrel_sb[:, l * D : (l + 1) * D],
                                  in_=p_wrel[l])
                nc.sync.dma_start(out=wroot_sb[:, l * D : (l + 1) * D],
                                  in_=p_wroot[l])
            w2_sb = singles.tile([P, 2 * OUT], F16)
            nc.sync.dma_start(out=w2_sb[:], in_=p_w2[:])
            bR_sb = singles.tile([1, L * D], F16)
            nc.sync.dma_start(out=bR_sb[:], in_=p_bR[:])
            ones_sb = singles.tile([1, P], F16)
            nc.vector.memset(ones_sb[:], 1.0)
            gammaT_sb = singles.tile([P, L], F32)
            nc.sync.dma_start(out=gammaT_sb[:], in_=p_gammaT[:])
            betaT_sb = singles.tile([P, L], F32)
            nc.sync.dma_start(out=betaT_sb[:], in_=p_betaT[:])
            b2_sb = singles.tile([1, OUT], F16)
            nc.sync.dma_start(out=b2_sb[:], in_=p_b2[:])
            eps_sb = singles.tile([P, 1], F32)
            nc.vector.memset(eps_sb[:], EPS)

            hT_prev = hT_pool.tile([P, SLOTS_PER_CORE], F16, tag="hT")
            nc.sync.dma_start(out=hT_prev[:], in_=p_xT[:])
            h_loc = p_xloc

            for l in range(L + 1):
                is_final = l == L
                rows = OUT if is_final else P
                if is_final:
                    zp0 = dram_zp.tile([N_CORES * rows, SLOTS_PER_CORE], F16)
                    zps = [zp0]
                else:
                    zps = [dram_zp.tile([N_CORES * rows, (b_ - a_) * P],
                                        F16, tag="zp_piece", name="zp_piece")
                           for (a_, b_) in PIECES]

                shard_buf = None
                zp_writes = []
                if not is_final:
                    zos = [dram_zo.tile([P, (b_ - a_) * P], F16,
                                        tag="zo_p", name="zo_p")
                           for (a_, b_) in PIECES]
                    piece_wr = [0] * len(PIECES)
                for (p0, nb, ntiles, co) in calls:
                    gath = g_pool.tile([P, ntiles * P], F16, tag="gath")
                    gg = gath.rearrange("p (t d) -> p t d", t=ntiles)
                    nc.gpsimd.dma_gather(
                        out_ap=gg,
                        in_ap=h_loc[:],
                        idxs_ap=idx_sb[:, co : co + ntiles * P // 16],
                        num_idxs=ntiles * P,
                        num_idxs_reg=call_regs[ntiles * P],
                        elem_size=D,
                        single_packet=False,
                    )
                    for pp in range(p0, p0 + nb):
                        b = int(proc_bins[pp])
                        t0 = int(tile_off[pp]) - int(tile_off[p0])
                        tb = int(TB[pp])
                        s_ = b // CHUNKS
                        j = b % CHUNKS
                        pi = next(i for i, (a_, b_) in enumerate(PIECES)
                                  if a_ <= j < b_)
                        pa, pb = PIECES[pi]
                        plen = pb - pa
                        jl = j - pa
                        gi = jl % 4
                        glen = min(4, plen - (jl - gi))
                        if jl == 0:
                            shard_buf = pair_pool.tile([rows, plen * P], F16)
                        sel = s_pool.tile([P, tb * P], F16)
                        for t in range(tb):
                            nc.vector.tensor_scalar(
                                out=sel[:, t * P : (t + 1) * P],
                                in0=iota_sb[:],
                                scalar1=dst_sb[
                                    :, tile_off[pp] + t : tile_off[pp] + t + 1
                                ],
                                scalar2=None,
                                op0=mybir.AluOpType.is_equal,
                            )
                        if gi == 0:
                            ps_a = psA.tile([P, glen * P], F32, space="PSUM")
                        for t in range(tb):
                            nc.tensor.matmul(
                                out=ps_a[:, gi * P : (gi + 1) * P],
                                lhsT=gath[:, (t0 + t) * P : (t0 + t + 1) * P],
                                rhs=sel[:, t * P : (t + 1) * P],
                                start=(t == 0),
                                stop=(t == tb - 1),
                            )
                        if is_final and gi == glen - 1:
                            agg_sb = aggo_pool.tile([P, glen * P], F16)
                            nc.scalar.activation(
                                out=agg_sb[:], in_=ps_a[:],
                                func=mybir.ActivationFunctionType.Copy,
                            )
                            ps2 = psZ.tile([OUT, glen * P], F32, space="PSUM", tag="psz")
                            for g2 in range(glen):
                                nc.tensor.matmul(
                                    out=ps2[:, g2 * P : (g2 + 1) * P],
                                    lhsT=w2_sb[:, :OUT],
                                    rhs=agg_sb[:, g2 * P : (g2 + 1) * P],
                                    start=True, stop=True,
                                )
                            nc.vector.tensor_copy(
                                out=shard_buf[:, (jl - gi) * P : (jl + 1) * P],
                                in_=ps2[:],
                            )
                        elif not is_final and gi == glen - 1:
                            if (jl // 4) % 2 == 0:
                                nc.scalar.activation(
                                    out=shard_buf[:, (jl - gi) * P : (jl + 1) * P],
                                    in_=ps_a[:],
                                    func=mybir.ActivationFunctionType.Copy,
                                )
                            else:
                                nc.vector.tensor_copy(
                                    out=shard_buf[:, (jl - gi) * P : (jl + 1) * P],
                                    in_=ps_a[:],
                                )
                        if jl == plen - 1:
                            if is_final:
                                dstap = bass.AP(
                                    tensor=zps[0].tensor,
                                    offset=zps[0][:].offset
                                    + (s_ * rows) * SLOTS_PER_CORE
                                    + pa * P,
                                    ap=[[SLOTS_PER_CORE, rows], [1, plen * P]],
                                )
                            else:
                                dstap = bass.AP(
                                    tensor=zps[pi].tensor,
                                    offset=zps[pi][:].offset
                                    + (s_ * rows) * (plen * P),
                                    ap=[[plen * P, rows], [1, plen * P]],
                                )
                            zp_w = nc.sync.dma_start(out=dstap,
                                                     in_=shard_buf[:])
                            zp_writes.append(zp_w)
                            if not is_final:
                                piece_wr[pi] += 1
                                if piece_wr[pi] == N_CORES:
                                    bass.BassGpSimd.collective_compute(
                                        nc.sync, "ReduceScatter",
                                        mybir.AluOpType.add,
                                        replica_groups=rg,
                                        ins=[zps[pi].opt()],
                                        outs=[zos[pi].opt()],
                                    )

                # ReduceScatter the partials (two pieces for inner layers:
                # RS of half A overlaps nothing here but runs while the tail
                # of half B compute / copies drain; RS_B follows on the
                # collective device)
                if is_final:
                    zo = dram_zo.tile([OUT, SLOTS_PER_CORE], F16)
                    nc.gpsimd.collective_compute(
                        "ReduceScatter", mybir.AluOpType.add,
                        replica_groups=rg, ins=[zps[0].opt()], outs=[zo.opt()],
                    )


                if is_final:
                    out16 = aggo_pool.tile([OUT, SLOTS_PER_CORE], F16,
                                           tag="aggo")
                    z2_sb = aggo_pool.tile([OUT, SLOTS_PER_CORE], F16,
                                           tag="aggo")
                    z2_ld = nc.sync.dma_start(out=z2_sb[:], in_=zo[:])
                    add_dep_helper(z2_ld.ins, zp_writes[-1].ins,
                                   reason="sp-order: z2 after zp writes")
                    for j in range(CHUNKS):
                        cs = slice(j * P, (j + 1) * P)
                        ps_z = psZ.tile([OUT, P], F32, space="PSUM", tag="psz")
                        nc.tensor.matmul(
                            out=ps_z[:], lhsT=w2_sb[:, OUT : 2 * OUT],
                            rhs=hT_prev[:, cs], start=True, stop=False,
                        )
                        nc.tensor.matmul(
                            out=ps_z[:], lhsT=b2_sb[:], rhs=ones_sb[:],
                            start=False, stop=True,
                        )
                        nc.scalar.activation(
                            out=out16[:, cs], in_=ps_z[:],
                            func=mybir.ActivationFunctionType.Copy,
                        )
                    nc.vector.tensor_tensor(
                        out=out16[:], in0=out16[:], in1=z2_sb[:],
                        op=mybir.AluOpType.add,
                    )
                    nc.sync.dma_start(out=p_out[:], in_=out16[:])
                    continue

                aggTs = []
                for pi, (a_, b_) in enumerate(PIECES):
                    t_agg = aggo_pool.tile([P, (b_ - a_) * P], F16,
                                           tag="aggo", name="aggT_p")
                    agg_ld = nc.sync.dma_start(out=t_agg[:], in_=zos[pi][:])
                    add_dep_helper(agg_ld.ins, zp_writes[-1].ins,
                                   reason="sp-order: aggT after zp writes")
                    aggTs.append(t_agg)
                z_all = z_pool.tile([P, SLOTS_PER_CORE], F16)
                stats = stat_pool.tile([P, CHUNKS, nc.vector.BN_STATS_DIM], F32)
                w_rel = wrel_sb[:, l * D : (l + 1) * D]
                w_root = wroot_sb[:, l * D : (l + 1) * D]
                for j in range(CHUNKS):
                    cs = slice(j * P, (j + 1) * P)
                    pi = next(i for i, (a_, b_) in enumerate(PIECES)
                              if a_ <= j < b_)
                    jrel = j - PIECES[pi][0]
                    agg_src = aggTs[pi][:, jrel * P : (jrel + 1) * P]
                    ps_z = psZ.tile([P, P], F32, space="PSUM", tag="psz")
                    nc.tensor.matmul(out=ps_z[:], lhsT=w_rel, rhs=agg_src,
                                     start=True, stop=False)
                    nc.tensor.matmul(out=ps_z[:], lhsT=w_root,
                                     rhs=hT_prev[:, cs], start=False,
                                     stop=False)
                    nc.tensor.matmul(
                        out=ps_z[:], lhsT=bR_sb[:, l * D : (l + 1) * D],
                        rhs=ones_sb[:], start=False, stop=True,
                    )
                    if j % 2 == 0:
                        nc.scalar.activation(
                            out=z_all[:, cs], in_=ps_z[:],
                            func=mybir.ActivationFunctionType.Copy,
                        )
                    else:
                        nc.vector.tensor_copy(out=z_all[:, cs], in_=ps_z[:])
                    nc.vector.bn_stats(
                        out=stats[:, j, :],
                        in_=z_all[:, j * P : j * P + widths[j]],
                    )

                # ---- BatchNorm over all nodes (mean of per-chunk widths
                # differs, so weight by width via two-group aggregation) ----
                bs = bn_pool.tile([P, 16], F32)
                mv = bs[:, 0:2]
                with tc.high_priority():
                    nc.vector.bn_aggr(out=mv, in_=stats[:, : CHUNKS - 1, :])
                mv2 = bs[:, 13:15]
                with tc.high_priority():
                    nc.vector.bn_aggr(out=mv2, in_=stats[:, CHUNKS - 1 :, :])
                # combine into sums: S1 = nf*mu1 + nl*mu2 ; S2 likewise
                nf = float((CHUNKS - 1) * FULL_W)
                nl = float(LAST_W)
                cc_sb = bs[:, 3:5]
                with tc.high_priority():
                    # cc0 = sum of z = nf*mu_f + nl*mu_l
                    nc.vector.tensor_scalar(
                        out=cc_sb[:, 0:1], in0=mv[:, 0:1], scalar1=nf,
                        scalar2=None, op0=mybir.AluOpType.mult,
                    )
                    nc.vector.tensor_scalar(
                        out=bs[:, 5:6], in0=mv2[:, 0:1], scalar1=nl,
                        scalar2=None, op0=mybir.AluOpType.mult,
                    )
                    nc.vector.tensor_tensor(
                        out=cc_sb[:, 0:1], in0=cc_sb[:, 0:1], in1=bs[:, 5:6],
                        op=mybir.AluOpType.add,
                    )
                    # cc1 = sum of z^2 = nf*(mu_f^2+var_f) + nl*(...)
                    nc.vector.tensor_scalar(
                        out=bs[:, 6:7], in0=mv[:, 0:1], scalar1=mv[:, 0:1],
                        scalar2=mv[:, 1:2], op0=mybir.AluOpType.mult,
                        op1=mybir.AluOpType.add,
                    )
                    nc.vector.tensor_scalar(
                        out=bs[:, 6:7], in0=bs[:, 6:7], scalar1=nf,
                        scalar2=None, op0=mybir.AluOpType.mult,
                    )
                    nc.vector.tensor_scalar(
                        out=bs[:, 7:8], in0=mv2[:, 0:1], scalar1=mv2[:, 0:1],
                        scalar2=mv2[:, 1:2], op0=mybir.AluOpType.mult,
                        op1=mybir.AluOpType.add,
                    )
                    nc.vector.tensor_scalar(
                        out=bs[:, 7:8], in0=bs[:, 7:8], scalar1=nl,
                        scalar2=None, op0=mybir.AluOpType.mult,
                    )
                    nc.vector.tensor_tensor(
                        out=cc_sb[:, 1:2], in0=bs[:, 6:7], in1=bs[:, 7:8],
                        op=mybir.AluOpType.add,
                    )
                cc_in = dram_cc.tile([P, 2], F32)
                cc_out = dram_cc.tile([P * N_CORES, 2], F32,
                                      addr_space="Shared")
                nc.sync.dma_start(out=cc_in[:], in_=cc_sb)
                nc.gpsimd.collective_compute(
                    "AllGather", mybir.AluOpType.bypass, replica_groups=rg,
                    ins=[cc_in.opt()], outs=[cc_out.opt()],
                )
                cc_all = bn_pool.tile([P, 2, N_CORES], F32)
                cc_src = bass.AP(
                    tensor=cc_out.tensor,
                    offset=cc_out[:].offset,
                    ap=[[2, P], [1, 2], [2 * P, N_CORES]],
                )
                nc.sync.dma_start(out=cc_all[:], in_=cc_src)
                cc_res = bs[:, 8:10]
                nc.vector.tensor_reduce(
                    out=cc_res.rearrange("p (a b) -> p a b", a=2),
                    in_=cc_all[:],
                    axis=mybir.AxisListType.X,
                    op=mybir.AluOpType.add,
                )
                mu = bs[:, 10:11]
                nc.vector.tensor_scalar(
                    out=mu, in0=cc_res[:, 0:1], scalar2=None,
                    op0=mybir.AluOpType.mult, scalar1=1.0 / N,
                )
                var = bs[:, 11:12]
                nc.vector.tensor_scalar(
                    out=var, in0=cc_res[:, 1:2], scalar2=None,
                    op0=mybir.AluOpType.mult, scalar1=1.0 / N,
                )
                mu2 = bs[:, 12:13]
                nc.vector.tensor_tensor(
                    out=mu2, in0=mu, in1=mu, op=mybir.AluOpType.mult
                )
                nc.vector.tensor_tensor(
                    out=var, in0=var, in1=mu2, op=mybir.AluOpType.subtract
                )
                rstd = bs[:, 13:14]
                nc.scalar.activation(
                    out=rstd, in_=var,
                    func=mybir.ActivationFunctionType.Sqrt,
                    bias=eps_sb[:], scale=1.0,
                )
                nc.vector.reciprocal(out=rstd, in_=rstd)
                scale = bs[:, 14:15]
                nc.vector.tensor_tensor(
                    out=scale, in0=rstd, in1=gammaT_sb[:, l : l + 1],
                    op=mybir.AluOpType.mult,
                )
                shift = bs[:, 15:16]
                nc.vector.tensor_tensor(
                    out=shift, in0=mu, in1=scale, op=mybir.AluOpType.mult
                )
                nc.vector.tensor_tensor(
                    out=shift, in0=betaT_sb[:, l : l + 1], in1=shift,
                    op=mybir.AluOpType.subtract,
                )

                # BN apply + relu (fp16 out), transpose, rebuild local table
                hT_new = hT_pool.tile([P, SLOTS_PER_CORE], F16, tag="hT")
                loc_new = dram_loc.tile([SLOTS_PER_CORE, D], F16)
                t16g = None
                for j in range(CHUNKS):
                    gi = j % 6
                    if gi == 0:
                        t16g = t16_pool.tile([P, 6, P], F16)
                        gs = slice(j * P, (j + 6) * P)
                        nc.scalar.activation(
                            out=hT_new[:, gs], in_=z_all[:, gs],
                            func=mybir.ActivationFunctionType.Relu,
                            bias=shift, scale=scale,
                        )
                    cs2 = slice(j * P, (j + 1) * P)
                    ps_t = psT.tile([P, P], F16, space="PSUM")
                    nc.tensor.transpose(
                        out=ps_t[:], in_=hT_new[:, cs2], identity=ident_sb[:],
                    )
                    if gi % 2 == 0:
                        nc.vector.tensor_copy(out=t16g[:, gi, :], in_=ps_t[:])
                    else:
                        nc.scalar.activation(
                            out=t16g[:, gi, :], in_=ps_t[:],
                            func=mybir.ActivationFunctionType.Copy,
                        )
                    if gi == 5:
                        g0 = (j - 5) * P
                        dstap = bass.AP(
                            tensor=loc_new.tensor,
                            offset=loc_new[:].offset + g0 * D,
                            ap=[[D, P], [P * D, 6], [1, D]],
                        )
                        nc.sync.dma_start(out=dstap, in_=t16g[:])
                h_loc = loc_new
                hT_prev = hT_new

    lower_extended_insts(nc)
    _split_multiwait(nc)
    return nc


_PROGRAM_CACHE = {}


def _get_program(meta):
    key = (meta["TILES"], meta["I_COLS"], tuple(meta["TB"].tolist()))
    if key not in _PROGRAM_CACHE:
        _PROGRAM_CACHE[key] = build_program(meta)
    return _PROGRAM_CACHE[key]


def _make_in_maps(meta, idx_cores, dst_cores, x_loc, xT_loc,
                  Wrel, Wroot, b, gamma, beta, Wrel2, Wroot2, b2):
    iota16 = np.broadcast_to(np.arange(P, dtype=np.float16), (P, P)).copy()
    ident16 = np.eye(P, dtype=np.float16)
    w2 = np.concatenate(
        [np.asarray(Wrel2, np.float32), np.asarray(Wroot2, np.float32)], axis=1
    ).astype(np.float16)
    common = dict(
        wrel=np.ascontiguousarray(np.asarray(Wrel, np.float32)).astype(np.float16),
        wroot=np.ascontiguousarray(np.asarray(Wroot, np.float32)).astype(np.float16),
        w2=np.ascontiguousarray(w2),
        bR=np.asarray(b, np.float32).reshape(1, L * D).astype(np.float16),
        gammaT=np.ascontiguousarray(np.asarray(gamma, np.float32).T),
        betaT=np.ascontiguousarray(np.asarray(beta, np.float32).T),
        b2=np.asarray(b2, np.float32).reshape(1, OUT).astype(np.float16),
        iota16=iota16,
        ident16=ident16,
    )
    in_maps = []
    for c in range(N_CORES):
        m = dict(common)
        m["x_loc"] = x_loc[c]
        m["xT_loc"] = xT_loc[c]
        m["gidx"] = idx_cores[c]
        m["dst_loc"] = dst_cores[c]
        in_maps.append(m)
    return in_maps


def run(x, edge_index, Wrel, Wroot, b, gamma, beta, Wrel2, Wroot2, b2):
    meta, newid, idx_cores, dst_cores, x_loc, xT_loc = _preprocess(
        x, edge_index)
    nc = _get_program(meta)
    in_maps = _make_in_maps(
        meta, idx_cores, dst_cores, x_loc, xT_loc,
        Wrel, Wroot, b, gamma, beta, Wrel2, Wroot2, b2,
    )
    from concourse.bass_utils import run_bass_kernel_spmd

    res = run_bass_kernel_spmd(nc, in_maps, list(range(N_CORES)))
    full = np.concatenate(
        [res.results[c]["z4T"].T for c in range(N_CORES)], axis=0
    )
    return full[newid].astype(np.float32), nc, meta["TILES"]


def kernel(**inputs):
    out, _, _ = run(**{k: np.asarray(v) for k, v in inputs.items()})
    return out



# revision 29
# speedup vs baseline: 1.0242x; 1.0067x over previous
"""GNN message-passing on 8 trn2 cores — source-sharded + ReduceScatter.

Strategy v2:
  - Nodes are partitioned across 8 cores x 54 chunks x 128 slots (<=116 real
    nodes per chunk, last chunk 102). A node's owner core holds BOTH its
    features (src role) and computes its BN/activation (dst role).
  - Edges are processed on the core that OWNS THE SOURCE: each core gathers
    only from its LOCAL node-major fp16 table (no replication!), reduces each
    global dst chunk with one-hot matmuls in PSUM, and stages fp16 partial
    aggregates [8 shards][128 feat][6912 slots] in DRAM.
  - One ReduceScatter per layer sums the partials and hands each core its own
    shard — out is only 1.77MB so the collective costs ~59us instead of the
    ~251us AllGather of the replicated-table design.
  - Dense transforms (agg@Wr + h@Ws + b) run post-RS on own chunks in fp16;
    BatchNorm stats via the tiny AllGather exchange; BN+ReLU fused on the
    scalar engine; PE transposes rebuild the local node-major table.
"""

import sys

import numpy as np

sys.path.insert(0, "/opt/trn_rl_repo")

import concourse.bass as bass  # noqa: E402
import concourse.mybir as mybir  # noqa: E402
import concourse.tile as tile  # noqa: E402
from concourse.vector_clock import ScopedClock  # noqa: E402
from concourse import library_config  # noqa: E402
from concourse.library_overlay import lower_extended_insts  # noqa: E402
from concourse.tile_rust import add_dep_helper  # noqa: E402

N = 50000
E = 800000
D = 128
L = 3
OUT = 2
EPS = 1e-5
N_CORES = 8
CHUNKS = 54                 # chunks (dst windows of 128 slots) per core
P = 128
FULL_W = 116                # real nodes in chunks 0..52
LAST_W = 102                # real nodes in chunk 53  (53*116 + 102 = 6250)
SLOTS_PER_CORE = CHUNKS * P          # 6912
N_PAD = N_CORES * SLOTS_PER_CORE     # 55296
NBINS = N_CORES * CHUNKS             # 432
GROUP_BINS = 18             # dst bins per dma_gather call
HALF = 27                   # chunks per RS half
PIECES = [(0, 27), (27, 46), (46, 54)]   # RS piece chunk ranges
CAP = 2 * P                 # target edges per (src core, bin)

F16 = mybir.dt.float16
F32 = mybir.dt.float32
I16 = mybir.dt.int16

_MAX_WAITS = 1


def _drain_and_barrier(self, tick_clock, wait_clock):
    nc = self.nc
    drain_inst = nc.sync.drain()
    wait_clock.add_sem_waits(
        drain_inst.ins, ScopedClock({None: tick_clock.global_clock})
    )
    si = drain_inst.ins.sync_info
    if si is not None and si.on_wait is not None and len(si.on_wait) > _MAX_WAITS:
        waits = list(si.on_wait)
        si.on_wait = waits[:_MAX_WAITS]
        rest = waits[_MAX_WAITS:]
        for i in range(0, len(rest), _MAX_WAITS):
            nop = nc.sync.nop(nofuse=True)
            nop.ins.sync_info = mybir.SyncInfo(
                on_wait=rest[i : i + _MAX_WAITS], on_update=[]
            )
    nc.all_engine_barrier()
    assert self.sems is not None
    popped = nc._tile_sem_poison_stack.pop()
    assert popped is self._sem_poison
    nc.clear_and_free_semaphores(list(self.sems.allocated().values()))
    nc.all_engine_barrier()


tile.TileContext._drain_and_barrier = _drain_and_barrier


def _split_multiwait(nc):
    for fn in nc.m.functions:
        for blk in fn.blocks:
            out = []
            for inst in blk.instructions:
                si = inst.sync_info
                if si is not None and si.on_wait and len(si.on_wait) > _MAX_WAITS:
                    waits = list(si.on_wait)
                    si.on_wait = waits[-_MAX_WAITS:]
                    rest = waits[:-_MAX_WAITS]
                    for i in range(0, len(rest), _MAX_WAITS):
                        out.append(
                            mybir.InstNoOp(
                                name=f"{inst.name}-ws{i}",
                                engine=inst.engine,
                                ins=[],
                                outs=[],
                                bass_nofuse=True,
                                sync_info=mybir.SyncInfo(
                                    on_wait=rest[i : i + _MAX_WAITS], on_update=[]
                                ),
                                debug=inst.debug,
                            )
                        )
                out.append(inst)
            blk.instructions[:] = out


# ---------------------------------------------------------------------------
# Host-side graph partitioning
# ---------------------------------------------------------------------------
def _bin_caps():
    caps = np.full(NBINS, FULL_W, np.int64)
    caps[CHUNKS - 1 :: CHUNKS] = LAST_W
    return caps


def _partition_nodes(src, dst):
    """Assign nodes to (bin, slot). Bins balance in-degree (LPT), then a
    repair pass swaps dst nodes between same-core bins so that per
    (src core, bin) edge counts stay <= CAP where possible."""
    rng = np.random.default_rng(7)
    deg = np.bincount(dst, minlength=N)
    caps = _bin_caps()
    order = np.argsort(-deg, kind="stable")
    bin_of = np.empty(N, np.int32)
    fill = np.zeros(NBINS, np.int64)
    sums = np.zeros(NBINS, np.int64)
    import heapq

    heap = [(0, b) for b in range(NBINS)]
    heapq.heapify(heap)
    for node in order:
        d = int(deg[node])
        while True:
            s, b = heapq.heappop(heap)
            if fill[b] < caps[b]:
                break
        bin_of[node] = b
        fill[b] += 1
        sums[b] += d
        if fill[b] < caps[b]:
            heapq.heappush(heap, (int(sums[b]), b))

    # repair: eliminate per (src core, bin) counts over CAP by swapping dst
    # nodes between same-core bins (global best-gain swaps, feasibility
    # checked vectorized over all candidate partners).
    core_of_node = bin_of // CHUNKS
    src_core = core_of_node[src]
    evc = np.zeros((N, N_CORES), np.int64)
    np.add.at(evc, (dst, src_core), 1)
    cnt = np.zeros((N_CORES, NBINS), np.int64)
    np.add.at(cnt, (src_core, bin_of[dst]), 1)
    order_n = np.argsort(bin_of, kind="stable")
    counts_b = np.bincount(bin_of, minlength=NBINS)
    ends_b = np.cumsum(counts_b)
    node_lists = [
        order_n[ends_b[b] - counts_b[b] : ends_b[b]].copy()
        for b in range(NBINS)
    ]
    import time as _time
    t_end = _time.time() + 60.0
    for _round in range(6):
        over = np.argwhere(cnt > CAP)
        if len(over) == 0 or _time.time() > t_end:
            break
        progress = False
        for c, b in over:
            core = b // CHUNKS
            binrange = np.arange(core * CHUNKS, (core + 1) * CHUNKS)
            while cnt[c, b] > CAP and _time.time() < t_end:
                lb = node_lists[b]
                vs = lb[np.argsort(-evc[lb, c])[:6]]
                best = None
                for v in vs:
                    vc = evc[v]
                    if vc[c] == 0:
                        break
                    for b2 in binrange:
                        if b2 == b:
                            continue
                        l2 = node_lists[b2]
                        wcs = evc[l2]
                        gain = vc[c] - wcs[:, c]
                        newb2 = cnt[:, b2][None] + vc[None] - wcs
                        newb = cnt[:, b][None] - vc[None] + wcs
                        ok = ((newb2 <= CAP).all(1)
                              & (newb <= np.maximum(CAP, cnt[:, b])[None])
                              .all(1) & (gain > 0))
                        if ok.any():
                            gm = np.where(ok, gain, -1)
                            i = int(np.argmax(gm))
                            cand = (int(gm[i]), v, b2, l2[i])
                            if best is None or cand[0] > best[0]:
                                best = cand
                if best is None:
                    break
                g, v, b2, w = best
                vc, wc = evc[v], evc[w]
                bin_of[v] = b2
                bin_of[w] = b
                cnt[:, b] += wc - vc
                cnt[:, b2] += vc - wc
                lb2 = node_lists[b2]
                node_lists[b] = np.where(node_lists[b] == v, w, node_lists[b])
                node_lists[b2] = np.where(lb2 == w, v, lb2)
                progress = True
        if not progress:
            break
    # slots within bin: real nodes first
    slot_of = np.empty(N, np.int32)
    for b in range(NBINS):
        nodes_b = np.where(bin_of == b)[0]
        slot_of[nodes_b] = np.arange(len(nodes_b))
    return bin_of, slot_of


def _preprocess(x, edge_index):
    x = np.asarray(x, np.float32)
    ei = np.asarray(edge_index)
    src = ei[0].astype(np.int64)
    dst = ei[1].astype(np.int64)
    bin_of, slot_of = _partition_nodes(src, dst)

    core_of_node = (bin_of // CHUNKS).astype(np.int64)
    chunk_of_node = (bin_of % CHUNKS).astype(np.int64)
    local_row = chunk_of_node * P + slot_of          # row in owner's table
    newid = core_of_node * SLOTS_PER_CORE + local_row

    src_core = core_of_node[src]
    e_bin = bin_of[dst].astype(np.int64)

    # processing order: half-major (chunks 0..26 of every shard first), so the
    # first ReduceScatter piece can overlap the second half's compute.
    proc_bins = np.array(
        [s_ * CHUNKS + j
         for (a, b_) in PIECES for s_ in range(N_CORES) for j in range(a, b_)],
        np.int64)
    pos_of_bin = np.empty(NBINS, np.int64)
    pos_of_bin[proc_bins] = np.arange(NBINS)

    # per (core, bin) counts -> tiles per bin (shared across cores)
    cnt = np.zeros((N_CORES, NBINS), np.int64)
    np.add.at(cnt, (src_core, e_bin), 1)
    TBraw = np.maximum(2, -(-cnt.max(axis=0) // P))  # [NBINS] by raw bin id
    TB = TBraw[proc_bins]                            # by processed position
    tile_off = np.concatenate([[0], np.cumsum(TB)[:-1]])  # by position
    TILES = int(TB.sum())

    # order edges per core by processed bin position
    idx_grids = np.zeros((N_CORES, TILES * P), np.int16)
    dst_grids = np.full((N_CORES, TILES * P), -1.0, np.float16)
    for c in range(N_CORES):
        m = src_core == c
        eb = pos_of_bin[e_bin[m]]
        es = local_row[src[m]]
        ed = slot_of[dst[m]]
        o = np.argsort(eb, kind="stable")
        eb, es, ed = eb[o], es[o], ed[o]
        starts = np.searchsorted(eb, np.arange(NBINS))
        pos_in_bin = np.arange(len(eb)) - starts[eb]
        flat = (tile_off[eb] * P) + pos_in_bin
        assert (pos_in_bin < TB[eb] * P).all()
        idx_grids[c, flat] = es.astype(np.int16)
        dst_grids[c, flat] = ed.astype(np.float16)

    # dst table [128, TILES]: value at (p, t) = dst slot of edge (t, p)
    dst_cores = np.ascontiguousarray(
        dst_grids.reshape(N_CORES, TILES, P).transpose(0, 2, 1)
    ).astype(np.float32)

    # gather calls: groups of GROUP_BINS bins; idx wrapped [i%16, i//16]
    # replicated to 128 partitions, columns contiguous per call.
    calls = []          # (first_pos, npos, ntiles, col_off)
    col_off = 0
    pos = 0
    for (a, b_) in PIECES:
        lim = pos + (b_ - a) * N_CORES
        b0 = pos
        while b0 < lim:
            nb = min(GROUP_BINS, lim - b0)
            ntiles = int(TB[b0 : b0 + nb].sum())
            calls.append((b0, nb, ntiles, col_off))
            col_off += ntiles * P // 16
            b0 += nb
        pos = lim
    I_COLS = col_off
    idx_cores = np.zeros((N_CORES, P, I_COLS), np.int16)
    for c in range(N_CORES):
        off = 0
        for (b0, nb, ntiles, co) in calls:
            t0 = int(tile_off[b0])
            seq = idx_grids[c, t0 * P : (t0 + ntiles) * P]
            w = seq.reshape(-1, 16).T            # [16, n/16]
            idx_cores[c, :, co : co + ntiles * P // 16] = np.tile(w, (8, 1))

    # initial tables
    x_pad = np.zeros((N_PAD, D), np.float32)
    x_pad[newid] = x
    x_loc = np.ascontiguousarray(
        x_pad.reshape(N_CORES, SLOTS_PER_CORE, D).astype(np.float16)
    )
    xT_loc = np.ascontiguousarray(
        x_pad.reshape(N_CORES, SLOTS_PER_CORE, D).transpose(0, 2, 1)
        .astype(np.float16)
    )
    meta = dict(TB=TB, tile_off=tile_off, TILES=TILES, calls=calls,
                I_COLS=I_COLS, proc_bins=proc_bins)
    return meta, newid, idx_cores, dst_cores, x_loc, xT_loc


# ---------------------------------------------------------------------------
# Device program
# ---------------------------------------------------------------------------
def build_program(meta):
    TB = meta["TB"]
    tile_off = meta["tile_off"]
    TILES = meta["TILES"]
    calls = meta["calls"]
    I_COLS = meta["I_COLS"]
    proc_bins = meta["proc_bins"]

    nc = bass.Bass(num_devices=N_CORES)

    p_xloc = nc.declare_dram_parameter("x_loc", [SLOTS_PER_CORE, D], F16,
                                       isOutput=False)
    p_xT = nc.declare_dram_parameter("xT_loc", [D, SLOTS_PER_CORE], F16,
                                     isOutput=False)
    p_idx = nc.declare_dram_parameter("gidx", [P, I_COLS], I16, isOutput=False)
    p_dst = nc.declare_dram_parameter("dst_loc", [P, TILES], F32,
                                      isOutput=False)
    p_wrel = nc.declare_dram_parameter("wrel", [L, D, D], F16, isOutput=False)
    p_wroot = nc.declare_dram_parameter("wroot", [L, D, D], F16, isOutput=False)
    p_w2 = nc.declare_dram_parameter("w2", [D, 2 * OUT], F16, isOutput=False)
    p_bR = nc.declare_dram_parameter("bR", [1, L * D], F16, isOutput=False)
    p_gammaT = nc.declare_dram_parameter("gammaT", [D, L], F32, isOutput=False)
    p_betaT = nc.declare_dram_parameter("betaT", [D, L], F32, isOutput=False)
    p_b2 = nc.declare_dram_parameter("b2", [1, OUT], F16, isOutput=False)
    p_iota = nc.declare_dram_parameter("iota16", [P, P], F16, isOutput=False)
    p_ident = nc.declare_dram_parameter("ident16", [P, P], F16, isOutput=False)
    p_out = nc.declare_dram_parameter("z4T", [OUT, SLOTS_PER_CORE], F16,
                                      isOutput=True)

    rg = [list(range(N_CORES))]
    widths = [FULL_W] * (CHUNKS - 1) + [LAST_W]

    from contextlib import ExitStack
    with tile.TileContext(nc) as tc:
        with ExitStack() as _es:
            dram_zp = _es.enter_context(tc.tile_pool(name="dram_zp", bufs=4, space="DRAM"))
            dram_zo = _es.enter_context(tc.tile_pool(name="dram_zo", bufs=4, space="DRAM"))
            dram_loc = _es.enter_context(tc.tile_pool(name="dram_loc", bufs=2, space="DRAM"))
            dram_cc = _es.enter_context(tc.tile_pool(name="dram_cc", bufs=2, space="DRAM"))
            singles = _es.enter_context(tc.tile_pool(name="singles", bufs=1))
            hT_pool = _es.enter_context(tc.tile_pool(name="hT", bufs=2))
            z_pool = _es.enter_context(tc.tile_pool(name="zb", bufs=1))
            g_pool = _es.enter_context(tc.tile_pool(name="gath", bufs=3))
            s_pool = _es.enter_context(tc.tile_pool(name="sel", bufs=8))
            pair_pool = _es.enter_context(tc.tile_pool(name="pairs", bufs=5))
            aggo_pool = _es.enter_context(tc.tile_pool(name="aggo", bufs=3))
            t16_pool = _es.enter_context(tc.tile_pool(name="t16p", bufs=2))
            bn_pool = _es.enter_context(tc.tile_pool(name="bns", bufs=2))
            stat_pool = _es.enter_context(tc.tile_pool(name="stat", bufs=2))
            psA = _es.enter_context(tc.tile_pool(name="psA", bufs=4, space="PSUM"))
            psZ = _es.enter_context(tc.tile_pool(name="psZ", bufs=2, space="PSUM"))
            psT = _es.enter_context(tc.tile_pool(name="psT", bufs=2, space="PSUM"))
            with tc.high_priority():
                nc.gpsimd.load_library(library_config.mlp)
            call_regs = {}
            for (_, _, ntiles, _) in calls:
                n = ntiles * P
                if n not in call_regs:
                    call_regs[n] = nc.gpsimd.to_reg(n)

            idx_sb = singles.tile([P, I_COLS], I16)
            _c3 = I_COLS // 8
            nc.sync.dma_start(out=idx_sb[:, :_c3], in_=p_idx[:, :_c3])
            nc.sync.dma_start(out=idx_sb[:, _c3 : 3 * _c3],
                              in_=p_idx[:, _c3 : 3 * _c3])
            nc.sync.dma_start(out=idx_sb[:, 3 * _c3 :], in_=p_idx[:, 3 * _c3 :])
            dst_sb = singles.tile([P, TILES], F32)
            nc.sync.dma_start(out=dst_sb[:], in_=p_dst[:])
            iota_sb = singles.tile([P, P], F16)
            nc.sync.dma_start(out=iota_sb[:], in_=p_iota[:])
            ident_sb = singles.tile([P, P], F16)
            nc.sync.dma_start(out=ident_sb[:], in_=p_ident[:])
            wrel_sb = singles.tile([P, L * D], F16)
            wroot_sb = singles.tile([P, L * D], F16)
            for l in range(L):
                nc.sync.dma_start(out=wrel_sb[:, l * D : (l + 1) * D],
                                  in_=p_wrel[l])
                nc.sync.dma_start(out=wroot_sb[:, l * D : (l + 1) * D],
                                  in_=p_wroot[l])
            w2_sb = singles.tile([P, 2 * OUT], F16)
            nc.sync.dma_start(out=w2_sb[:], in_=p_w2[:])
            bR_sb = singles.tile([1, L * D], F16)
            nc.sync.dma_start(out=bR_sb[:], in_=p_bR[:])
            ones_sb = singles.tile([1, P], F16)
            nc.vector.memset(ones_sb[:], 1.0)
            gammaT_sb = singles.tile([P, L], F32)
            nc.sync.dma_start(out=gammaT_sb[:], in_=p_gammaT[:])
            betaT_sb = singles.tile([P, L], F32)
            nc.sync.dma_start(out=betaT_sb[:], in_=p_betaT[:])
            b2_sb = singles.tile([1, OUT], F16)
            nc.sync.dma_start(out=b2_sb[:], in_=p_b2[:])
            eps_sb = singles.tile([P, 1], F32)
            nc.vector.memset(eps_sb[:], EPS)

            hT_prev = hT_pool.tile([P, SLOTS_PER_CORE], F16, tag="hT")
            nc.sync.dma_start(out=hT_prev[:], in_=p_xT[:])
            h_loc = p_xloc

            for l in range(L + 1):
                is_final = l == L
                rows = OUT if is_final else P
                if is_final:
                    zp0 = dram_zp.tile([N_CORES * rows, SLOTS_PER_CORE], F16)
                    zps = [zp0]
                else:
                    zps = [dram_zp.tile([N_CORES * rows, (b_ - a_) * P],
                                        F16, tag="zp_piece", name="zp_piece")
                           for (a_, b_) in PIECES]

                shard_buf = None
                zp_writes = []
                if not is_final:
                    zos = [dram_zo.tile([P, (b_ - a_) * P], F16,
                                        tag="zo_p", name="zo_p")
                           for (a_, b_) in PIECES]
                    piece_wr = [0] * len(PIECES)
                for (p0, nb, ntiles, co) in calls:
                    gath = g_pool.tile([P, ntiles * P], F16, tag="gath")
                    gg = gath.rearrange("p (t d) -> p t d", t=ntiles)
                    nc.gpsimd.dma_gather(
                        out_ap=gg,
                        in_ap=h_loc[:],
                        idxs_ap=idx_sb[:, co : co + ntiles * P // 16],
                        num_idxs=ntiles * P,
                        num_idxs_reg=call_regs[ntiles * P],
                        elem_size=D,
                        single_packet=False,
                    )
                    for pp in range(p0, p0 + nb):
                        b = int(proc_bins[pp])
                        t0 = int(tile_off[pp]) - int(tile_off[p0])
                        tb = int(TB[pp])
                        s_ = b // CHUNKS
                        j = b % CHUNKS
                        pi = next(i for i, (a_, b_) in enumerate(PIECES)
                                  if a_ <= j < b_)
                        pa, pb = PIECES[pi]
                        plen = pb - pa
                        jl = j - pa
                        gi = jl % 4
                        glen = min(4, plen - (jl - gi))
                        if jl == 0:
                            shard_buf = pair_pool.tile([rows, plen * P], F16)
                        sel = s_pool.tile([P, tb * P], F16)
                        for t in range(tb):
                            nc.vector.tensor_scalar(
                                out=sel[:, t * P : (t + 1) * P],
                                in0=iota_sb[:],
                                scalar1=dst_sb[
                                    :, tile_off[pp] + t : tile_off[pp] + t + 1
                                ],
                                scalar2=None,
                                op0=mybir.AluOpType.is_equal,
                            )
                        if gi == 0:
                            ps_a = psA.tile([P, glen * P], F32, space="PSUM")
                        for t in range(tb):
                            nc.tensor.matmul(
                                out=ps_a[:, gi * P : (gi + 1) * P],
                                lhsT=gath[:, (t0 + t) * P : (t0 + t + 1) * P],
                                rhs=sel[:, t * P : (t + 1) * P],
                                start=(t == 0),
                                stop=(t == tb - 1),
                            )
                        if is_final and gi == glen - 1:
                            agg_sb = aggo_pool.tile([P, glen * P], F16)
                            nc.scalar.activation(
                                out=agg_sb[:], in_=ps_a[:],
                                func=mybir.ActivationFunctionType.Copy,
                            )
                            ps2 = psZ.tile([OUT, glen * P], F32, space="PSUM", tag="psz")
                            for g2 in range(glen):
                                nc.tensor.matmul(
                                    out=ps2[:, g2 * P : (g2 + 1) * P],
                                    lhsT=w2_sb[:, :OUT],
                                    rhs=agg_sb[:, g2 * P : (g2 + 1) * P],
                                    start=True, stop=True,
                                )
                            nc.vector.tensor_copy(
                                out=shard_buf[:, (jl - gi) * P : (jl + 1) * P],
                                in_=ps2[:],
                            )
                        elif not is_final and gi == glen - 1:
                            if (jl // 4) % 2 == 0:
                                nc.scalar.activation(
                                    out=shard_buf[:, (jl - gi) * P : (jl + 1) * P],
                                    in_=ps_a[:],
                                    func=mybir.ActivationFunctionType.Copy,
                                )
                            else:
                                nc.vector.tensor_copy(
                                    out=shard_buf[:, (jl - gi) * P : (jl + 1) * P],
                                    in_=ps_a[:],
                                )
                        if jl == plen - 1:
                            if is_final:
                                dstap = bass.AP(
                                    tensor=zps[0].tensor,
                                    offset=zps[0][:].offset
                                    + (s_ * rows) * SLOTS_PER_CORE
                                    + pa * P,
                                    ap=[[SLOTS_PER_CORE, rows], [1, plen * P]],
                                )
                            else:
                                dstap = bass.AP(
                                    tensor=zps[pi].tensor,
                                    offset=zps[pi][:].offset
                                    + (s_ * rows) * (plen * P),
                                    ap=[[plen * P, rows], [1, plen * P]],
                                )
                            zp_w = nc.sync.dma_start(out=dstap,
                                                     in_=shard_buf[:])
                            zp_writes.append(zp_w)
                            if not is_final:
                                piece_wr[pi] += 1
                                if piece_wr[pi] == N_CORES:
                                    bass.BassGpSimd.collective_compute(
                                        nc.sync, "ReduceScatter",
                                        mybir.AluOpType.add,
                                        replica_groups=rg,
                                        ins=[zps[pi].opt()],
                                        outs=[zos[pi].opt()],
                                    )

                # ReduceScatter the partials (two pieces for inner layers:
                # RS of half A overlaps nothing here but runs while the tail
                # of half B compute / copies drain; RS_B follows on the
                # collective device)
                if is_final:
                    zo = dram_zo.tile([OUT, SLOTS_PER_CORE], F16)
                    nc.gpsimd.collective_compute(
                        "ReduceScatter", mybir.AluOpType.add,
                        replica_groups=rg, ins=[zps[0].opt()], outs=[zo.opt()],
                    )


                if is_final:
                    out16 = aggo_pool.tile([OUT, SLOTS_PER_CORE], F16,
                                           tag="aggo")
                    z2_sb = aggo_pool.tile([OUT, SLOTS_PER_CORE], F16,
                                           tag="aggo")
                    z2_ld = nc.sync.dma_start(out=z2_sb[:], in_=zo[:])
                    add_dep_helper(z2_ld.ins, zp_writes[-1].ins,
                                   reason="sp-order: z2 after zp writes")
                    for j in range(CHUNKS):
                        cs = slice(j * P, (j + 1) * P)
                        ps_z = psZ.tile([OUT, P], F32, space="PSUM", tag="psz")
                        nc.tensor.matmul(
                            out=ps_z[:], lhsT=w2_sb[:, OUT : 2 * OUT],
                            rhs=hT_prev[:, cs], start=True, stop=False,
                        )
                        nc.tensor.matmul(
                            out=ps_z[:], lhsT=b2_sb[:], rhs=ones_sb[:],
                            start=False, stop=True,
                        )
                        nc.scalar.activation(
                            out=out16[:, cs], in_=ps_z[:],
                            func=mybir.ActivationFunctionType.Copy,
                        )
                    nc.vector.tensor_tensor(
                        out=out16[:], in0=out16[:], in1=z2_sb[:],
                        op=mybir.AluOpType.add,
                    )
                    nc.sync.dma_start(out=p_out[:], in_=out16[:])
                    continue

                aggTs = []
                for pi, (a_, b_) in enumerate(PIECES):
                    t_agg = aggo_pool.tile([P, (b_ - a_) * P], F16,
                                           tag="aggo", name="aggT_p")
                    agg_ld = nc.sync.dma_start(out=t_agg[:], in_=zos[pi][:])
                    add_dep_helper(agg_ld.ins, zp_writes[-1].ins,
                                   reason="sp-order: aggT after zp writes")
                    aggTs.append(t_agg)
                z_all = z_pool.tile([P, SLOTS_PER_CORE], F16)
                stats = stat_pool.tile([P, CHUNKS, nc.vector.BN_STATS_DIM], F32)
                w_rel = wrel_sb[:, l * D : (l + 1) * D]
                w_root = wroot_sb[:, l * D : (l + 1) * D]
                for j in range(CHUNKS):
                    cs = slice(j * P, (j + 1) * P)
                    pi = next(i for i, (a_, b_) in enumerate(PIECES)
                              if a_ <= j < b_)
                    jrel = j - PIECES[pi][0]
                    agg_src = aggTs[pi][:, jrel * P : (jrel + 1) * P]
                    ps_z = psZ.tile([P, P], F32, space="PSUM", tag="psz")
                    nc.tensor.matmul(out=ps_z[:], lhsT=w_rel, rhs=agg_src,
                                     start=True, stop=False)
                    nc.tensor.matmul(out=ps_z[:], lhsT=w_root,
                                     rhs=hT_prev[:, cs], start=False,
                                     stop=False)
                    nc.tensor.matmul(
                        out=ps_z[:], lhsT=bR_sb[:, l * D : (l + 1) * D],
                        rhs=ones_sb[:], start=False, stop=True,
                    )
                    if j % 2 == 0:
                        nc.scalar.activation(
                            out=z_all[:, cs], in_=ps_z[:],
                            func=mybir.ActivationFunctionType.Copy,
                        )
                    else:
                        nc.vector.tensor_copy(out=z_all[:, cs], in_=ps_z[:])
                    nc.vector.bn_stats(
                        out=stats[:, j, :],
                        in_=z_all[:, j * P : j * P + widths[j]],
                    )

                # ---- BatchNorm over all nodes (mean of per-chunk widths
                # differs, so weight by width via two-group aggregation) ----
                bs = bn_pool.tile([P, 16], F32)
                mv = bs[:, 0:2]
                with tc.high_priority():
                    nc.vector.bn_aggr(out=mv, in_=stats[:, : CHUNKS - 1, :])
                mv2 = bs[:, 13:15]
                with tc.high_priority():
                    nc.vector.bn_aggr(out=mv2, in_=stats[:, CHUNKS - 1 :, :])
                # combine into sums: S1 = nf*mu1 + nl*mu2 ; S2 likewise
                nf = float((CHUNKS - 1) * FULL_W)
                nl = float(LAST_W)
                cc_sb = bs[:, 3:5]
                with tc.high_priority():
                    # cc0 = sum of z = nf*mu_f + nl*mu_l
                    nc.vector.tensor_scalar(
                        out=cc_sb[:, 0:1], in0=mv[:, 0:1], scalar1=nf,
                        scalar2=None, op0=mybir.AluOpType.mult,
                    )
                    nc.vector.tensor_scalar(
                        out=bs[:, 5:6], in0=mv2[:, 0:1], scalar1=nl,
                        scalar2=None, op0=mybir.AluOpType.mult,
                    )
                    nc.vector.tensor_tensor(
                        out=cc_sb[:, 0:1], in0=cc_sb[:, 0:1], in1=bs[:, 5:6],
                        op=mybir.AluOpType.add,
                    )
                    # cc1 = sum of z^2 = nf*(mu_f^2+var_f) + nl*(...)
                    nc.vector.tensor_scalar(
                        out=bs[:, 6:7], in0=mv[:, 0:1], scalar1=mv[:, 0:1],
                        scalar2=mv[:, 1:2], op0=mybir.AluOpType.mult,
                        op1=mybir.AluOpType.add,
                    )
                    nc.vector.tensor_scalar(
                        out=bs[:, 6:7], in0=bs[:, 6:7], scalar1=nf,
                        scalar2=None, op0=mybir.AluOpType.mult,
                    )
                    nc.vector.tensor_scalar(
                        out=bs[:, 7:8], in0=mv2[:, 0:1], scalar1=mv2[:, 0:1],
                        scalar2=mv2[:, 1:2], op0=mybir.AluOpType.mult,
                        op1=mybir.AluOpType.add,
                    )
                    nc.vector.tensor_scalar(
                        out=bs[:, 7:8], in0=bs[:, 7:8], scalar1=nl,
                        scalar2=None, op0=mybir.AluOpType.mult,
                    )
                    nc.vector.tensor_tensor(
                        out=cc_sb[:, 1:2], in0=bs[:, 6:7], in1=bs[:, 7:8],
                        op=mybir.AluOpType.add,
                    )
                cc_in = dram_cc.tile([P, 2], F32)
                cc_out = dram_cc.tile([P * N_CORES, 2], F32,
                                      addr_space="Shared")
                nc.sync.dma_start(out=cc_in[:], in_=cc_sb)
                nc.gpsimd.collective_compute(
                    "AllGather", mybir.AluOpType.bypass, replica_groups=rg,
                    ins=[cc_in.opt()], outs=[cc_out.opt()],
                )
                cc_all = bn_pool.tile([P, 2, N_CORES], F32)
                cc_src = bass.AP(
                    tensor=cc_out.tensor,
                    offset=cc_out[:].offset,
                    ap=[[2, P], [1, 2], [2 * P, N_CORES]],
                )
                nc.sync.dma_start(out=cc_all[:], in_=cc_src)
                cc_res = bs[:, 8:10]
                nc.vector.tensor_reduce(
                    out=cc_res.rearrange("p (a b) -> p a b", a=2),
                    in_=cc_all[:],
                    axis=mybir.AxisListType.X,
                    op=mybir.AluOpType.add,
                )
                mu = bs[:, 10:11]
                nc.vector.tensor_scalar(
                    out=mu, in0=cc_res[:, 0:1], scalar2=None,
                    op0=mybir.AluOpType.mult, scalar1=1.0 / N,
                )
                var = bs[:, 11:12]
                nc.vector.tensor_scalar(
                    out=var, in0=cc_res[:, 1:2], scalar2=None,
                    op0=mybir.AluOpType.mult, scalar1=1.0 / N,
                )
                mu2 = bs[:, 12:13]
                nc.vector.tensor_tensor(
                    out=mu2, in0=mu, in1=mu, op=mybir.AluOpType.mult
                )
                nc.vector.tensor_tensor(
                    out=var, in0=var, in1=mu2, op=mybir.AluOpType.subtract
                )
                rstd = bs[:, 13:14]
                nc.scalar.activation(
                    out=rstd, in_=var,
                    func=mybir.ActivationFunctionType.Sqrt,
                    bias=eps_sb[:], scale=1.0,
                )
                nc.vector.reciprocal(out=rstd, in_=rstd)
                scale = bs[:, 14:15]
                nc.vector.tensor_tensor(
                    out=scale, in0=rstd, in1=gammaT_sb[:, l : l + 1],
                    op=mybir.AluOpType.mult,
                )
                shift = bs[:, 15:16]
                nc.vector.tensor_tensor(
                    out=shift, in0=mu, in1=scale, op=mybir.AluOpType.mult
                )
                nc.vector.tensor_tensor(
                    out=shift, in0=betaT_sb[:, l : l + 1], in1=shift,
                    op=mybir.AluOpType.subtract,
                )

                # BN apply + relu (fp16 out), transpose, rebuild local table
                hT_new = hT_pool.tile([P, SLOTS_PER_CORE], F16, tag="hT")
                loc_new = dram_loc.tile([SLOTS_PER_CORE, D], F16)
                t16g = None
                for j in range(CHUNKS):
                    gi = j % 6
                    if gi == 0:
                        t16g = t16_pool.tile([P, 6, P], F16)
                        gs = slice(j * P, (j + 6) * P)
                        nc.scalar.activation(
                            out=hT_new[:, gs], in_=z_all[:, gs],
                            func=mybir.ActivationFunctionType.Relu,
                            bias=shift, scale=scale,
                        )
                    cs2 = slice(j * P, (j + 1) * P)
                    ps_t = psT.tile([P, P], F16, space="PSUM")
                    nc.tensor.transpose(
                        out=ps_t[:], in_=hT_new[:, cs2], identity=ident_sb[:],
                    )
                    nc.vector.tensor_copy(out=t16g[:, gi, :], in_=ps_t[:])
                    if gi == 5:
                        g0 = (j - 5) * P
                        dstap = bass.AP(
                            tensor=loc_new.tensor,
                            offset=loc_new[:].offset + g0 * D,
                            ap=[[D, P], [P * D, 6], [1, D]],
                        )
                        nc.sync.dma_start(out=dstap, in_=t16g[:])
                h_loc = loc_new
                hT_prev = hT_new

    lower_extended_insts(nc)
    _split_multiwait(nc)
    return nc


_PROGRAM_CACHE = {}


def _get_program(meta):
    key = (meta["TILES"], meta["I_COLS"], tuple(meta["TB"].tolist()))
    if key not in _PROGRAM_CACHE:
        _PROGRAM_CACHE[key] = build_program(meta)
    return _PROGRAM_CACHE[key]


def _make_in_maps(meta, idx_cores, dst_cores, x_loc, xT_loc,
                  Wrel, Wroot, b, gamma, beta, Wrel2, Wroot2, b2):
    iota16 = np.broadcast_to(np.arange(P, dtype=np.float16), (P, P)).copy()
    ident16 = np.eye(P, dtype=np.float16)
    w2 = np.concatenate(
        [np.asarray(Wrel2, np.float32), np.asarray(Wroot2, np.float32)], axis=1
    ).astype(np.float16)
    common = dict(
        wrel=np.ascontiguousarray(np.asarray(Wrel, np.float32)).astype(np.float16),
        wroot=np.ascontiguousarray(np.asarray(Wroot, np.float32)).astype(np.float16),
        w2=np.ascontiguousarray(w2),
        bR=np.asarray(b, np.float32).reshape(1, L * D).astype(np.float16),
        gammaT=np.ascontiguousarray(np.asarray(gamma, np.float32).T),
        betaT=np.ascontiguousarray(np.asarray(beta, np.float32).T),
        b2=np.asarray(b2, np.float32).reshape(1, OUT).astype(np.float16),
        iota16=iota16,
        ident16=ident16,
    )
    in_maps = []
    for c in range(N_CORES):
        m = dict(common)
        m["x_loc"] = x_loc[c]
        m["xT_loc"] = xT_loc[c]
        m["gidx"] = idx_cores[c]
        m["dst_loc"] = dst_cores[c]
        in_maps.append(m)
    return in_maps


def run(x, edge_index, Wrel, Wroot, b, gamma, beta, Wrel2, Wroot2, b2):
    meta, newid, idx_cores, dst_cores, x_loc, xT_loc = _preprocess(
        x, edge_index)
    nc = _get_program(meta)
    in_maps = _make_in_maps(
        meta, idx_cores, dst_cores, x_loc, xT_loc,
        Wrel, Wroot, b, gamma, beta, Wrel2, Wroot2, b2,
    )
    from concourse.bass_utils import run_bass_kernel_spmd

    res = run_bass_kernel_spmd(nc, in_maps, list(range(N_CORES)))
    full = np.concatenate(
        [res.results[c]["z4T"].T for c in range(N_CORES)], axis=0
    )
    return full[newid].astype(np.float32), nc, meta["TILES"]


def kernel(**inputs):
    out, _, _ = run(**{k: np.asarray(v) for k, v in inputs.items()})
    return out

